# revision 37
# baseline (speedup 1.0000x reference)
"""Banded (sliding-window) attention kernel for Trainium2, 8 NeuronCores.

Problem: nn_AttentionLAI (B=2, N=4096, C=384, H=6, head_dim=64), epoch=0
=> band window w=8 (each query attends keys with |i-j| <= 8).

Sharding: sequence sharding. core c = b*4 + s handles batch b, rows
[s*1024, (s+1)*1024) with a w-row halo on each side.  Zero collectives;
host gathers the 8 per-core outputs.

PSUM discipline (empirically validated on HW): at most ONE matmul output
column-range per 2KB bank at a time; writes at different partition
offsets to that same column range are fine; sequential reuse of a bank
with a different range is fine once prior readers complete.  The whole
PSUM is managed as one [128, 4096] f32 tile with manual bank regions.

Per-core dataflow (fp16 operands -> f32 PSUM accumulate):
  xT   [384, RT]   x-shard transposed (feature-major), RT = 1024 + 2w
  qkT  [768, RT]   = qk_w^T-chunks @ xT     (feature-major Q^T / K^T)
  V    [RT, 384]   = xT-chunks^T @ v_w^T    (row-major V)
  per 128-query block b (8 blocks):
    S_h  = Q^T_h' K^T_hslab -> PSUM bank h            (+ edge kmask acc)
    E    = exp(S)  (one ACT op, strided across banks)
    P    = E * band_mask;  sums = rowsum(P);  Phat = P / sums  (DVE)
    PT   = transpose(Phat) -> banks 0-5 main, 6-7 + 0-3 tails; bounce
    O^T  = V^T-chunks: 12 matmuls -> banks 4-6 (2 heads/bank partition-
           packed); bounce -> aT (feature-major, ready for projection)
    Y    = aT^T @ proj_w^T (+bias) -> bank 7; bounce; DMA out rows.
"""

import numpy as np

B, N, C, H = 2, 4096, 384, 6
HD = C // H            # 64
SCALE = HD ** -0.5
MASK_EPOCHS = [10, 20, 30, 40]
NCORES = 8
SEQ_SHARDS = 4         # per batch
R = N // SEQ_SHARDS    # 1024 rows per core
NBLK = R // 128        # 8 query blocks per core
NEG = -60000.0         # additive mask value (fits fp16, exp() -> 0)


def _window_for_epoch(epoch):
    if epoch >= MASK_EPOCHS[-1]:
        return None
    if epoch < MASK_EPOCHS[-4]:
        return 8
    elif epoch < MASK_EPOCHS[-3]:
        return 12
    elif epoch < MASK_EPOCHS[-2]:
        return 18
    else:
        return 20


def _numpy_fallback(x, qkv_w, proj_w, proj_b, w):
    """Reference-equivalent host computation (used only for epoch>=40)."""
    b, n, c = x.shape
    qkv = (x @ qkv_w.T).reshape(b, n, 3, H, HD).transpose(2, 0, 3, 1, 4)
    q, k, v = qkv[0], qkv[1], qkv[2]
    attn = np.einsum("bhnd,bhmd->bhnm", q, k) * SCALE
    if w is not None:
        idx = np.arange(n)
        band = np.abs(idx[:, None] - idx[None, :]) <= w
        attn = np.where(band[None, None], attn, -1e9)
    attn = attn - attn.max(axis=-1, keepdims=True)
    attn = np.exp(attn)
    attn = attn / attn.sum(axis=-1, keepdims=True)
    out = np.einsum("bhnm,bhmd->bhnd", attn, v)
    out = out.transpose(0, 2, 1, 3).reshape(b, n, c)
    return (out @ proj_w.T + proj_b).astype(np.float32)


_PROGRAM_CACHE = {}


def build_program(w, with_bias):
    """Build the SPMD Bass program for band window w. Returns nc."""
    import concourse.bass as bass
    import concourse.mybir as mybir
    import concourse.tile as tile
    from concourse import bacc
    from concourse.tile import add_dep_helper
    from contextlib import ExitStack

    f16 = mybir.dt.float16
    f32 = mybir.dt.float32
    AF = mybir.ActivationFunctionType
    AX = mybir.AxisListType

    W2 = 2 * w
    SLAB = 128 + W2            # keys per 128-query block
    RT = R + W2                # haloed rows per core
    assert SLAB <= 512
    NCH = [(0, 512), (512, 512), (1024, RT - 1024)]  # qkT col chunks
    NVCH = RT // 128 + (1 if RT % 128 else 0)        # V row chunks (9)

    nc = bacc.Bacc()

    xT_d = nc.declare_dram_parameter("xT", [C, RT], f16, isOutput=False)
    qkw_d = nc.declare_dram_parameter("qk_wT", [C, 2 * C], f16, isOutput=False)
    vw_d = nc.declare_dram_parameter("v_wT", [C, C], f16, isOutput=False)
    pw_d = nc.declare_dram_parameter("proj_wT", [C, C], f16, isOutput=False)
    band_d = nc.declare_dram_parameter("band", [128, 6 * SLAB], f16, isOutput=False)
    kmask_d = nc.declare_dram_parameter("kmask", [1, RT], f16, isOutput=False)
    ident_d = nc.declare_dram_parameter("ident", [128, 128], f16, isOutput=False)
    if with_bias:
        pb_d = nc.declare_dram_parameter("proj_b16", [1, C], f16, isOutput=False)
    out_d = nc.declare_dram_parameter("out", [R, C], f16, isOutput=True)

    def apx(sl, dims, extra_off=0):
        """AP with custom free dims on top of a slice's partition dim."""
        return bass.AP(tensor=sl.tensor, offset=sl.offset + extra_off,
                       ap=[sl.ap[0]] + dims)

    with ExitStack() as ctx:
        tc = ctx.enter_context(tile.TileContext(nc))
        consts = ctx.enter_context(tc.tile_pool(name="consts", bufs=1))

        xT = consts.tile([128, 3, RT], f16)
        qkw = consts.tile([128, 3, 2 * C], f16)
        vw = consts.tile([128, 3, C], f16)
        pw = consts.tile([128, 3, C], f16)
        band = consts.tile([128, 6, SLAB], f16)
        kmask = consts.tile([1, RT], f16)
        ident = consts.tile([128, 128], f16)
        ones1 = consts.tile([1, 128], f16)
        nc.vector.memset(ones1, 1.0)
        if with_bias:
            pb = consts.tile([1, C], f16)
            nc.sync.dma_start(out=pb, in_=pb_d[:, :])
        # spread input DMAs over four queues; phase-1's first deps come first
        half = RT // 2
        for j in range(3):
            nc.sync.dma_start(out=xT[:, j, 0:half],
                              in_=xT_d[128 * j:128 * (j + 1), 0:half])
            nc.scalar.dma_start(out=xT[:, j, half:RT],
                                in_=xT_d[128 * j:128 * (j + 1), half:RT])
            nc.gpsimd.dma_start(out=qkw[:, j, :], in_=qkw_d[128 * j:128 * (j + 1), :])
        for j in range(3):
            nc.gpsimd.dma_start(out=vw[:, j, :], in_=vw_d[128 * j:128 * (j + 1), :])
            nc.scalar.dma_start(out=pw[:, j, :], in_=pw_d[128 * j:128 * (j + 1), :])
        nc.scalar.dma_start(out=band[:, :, :], in_=band_d[:, :])
        nc.gpsimd.dma_start(out=kmask, in_=kmask_d[:, :])
        nc.gpsimd.dma_start(out=ident, in_=ident_d[:, :])

        qkT = consts.tile([128, 6, RT], f16)
        V = consts.tile([128, NVCH, C], f16)
        # ping/pong normalized-score buffers; slot padded to 256 so the
        # 16-wide tail can be DMA-transposed as a full 128-col window
        Phb = []
        for i in range(2):
            Phx = consts.tile([128, 6, 256], f16, tag="Phb%d" % i)
            nc.gpsimd.memset(Phx[:, :, SLAB:256], 0.0)
            Phb.append(Phx)

        # ---- phase 1: qkT = qk_w^T @ x^T  (feature-major) ----
        eng = 0
        with ExitStack() as ph1:
            qkps = ph1.enter_context(tc.tile_pool(name="qkps", bufs=6, space="PSUM"))
            for (n0, ns) in NCH:
                for j in range(6):
                    ps = qkps.tile([128, 512], f32, tag="qkps")
                    for cc in range(3):
                        nc.tensor.matmul(
                            ps[:, :ns],
                            lhsT=qkw[:, cc, 128 * j:128 * (j + 1)],
                            rhs=xT[:, cc, n0:n0 + ns],
                            start=(cc == 0), stop=(cc == 2))
                    if eng % 2 == 0:
                        nc.vector.tensor_copy(qkT[:, j, n0:n0 + ns], ps[:, :ns])
                    else:
                        nc.scalar.copy(qkT[:, j, n0:n0 + ns], ps[:, :ns])
                    eng += 1
            # ---- phase 2: V = (x^T)^T @ v_w^T  (row-major) ----
            for r in range(NVCH):
                rlen = min(128, RT - 128 * r)
                ps = qkps.tile([128, 512], f32, tag="qkps")
                for cc in range(3):
                    nc.tensor.matmul(
                        ps[:rlen, :C],
                        lhsT=xT[:, cc, 128 * r:128 * r + rlen],
                        rhs=vw[:, cc, :],
                        start=(cc == 0), stop=(cc == 2))
                if eng % 2 == 0:
                    nc.vector.tensor_copy(V[:rlen, r, :], ps[:rlen, :C])
                else:
                    nc.scalar.copy(V[:rlen, r, :], ps[:rlen, :C])
                eng += 1


        # ---- phase 3: per-block attention + projection ----
        pspool = ctx.enter_context(tc.tile_pool(name="ps8", bufs=1, space="PSUM"))
        PS = pspool.tile([128, 4096], f32)          # the whole PSUM
        ps16 = PS[:, :].bitcast(f16)                # [128, 8192] fp16 view
        sbuf = ctx.enter_context(tc.tile_pool(name="work", bufs=2))

        tiles = {}

        def emit_scores(b):
            q0 = w + 128 * b
            k0 = 128 * b
            edge = b in (0, NBLK - 1)
            for h in range(6):
                nc.tensor.matmul(
                    PS[:, 512 * h:512 * h + SLAB],
                    lhsT=qkT[64 * (h % 2):64 * (h % 2) + 64, h // 2, q0:q0 + 128],
                    rhs=qkT[64 * (h % 2):64 * (h % 2) + 64, 3 + h // 2, k0:k0 + SLAB],
                    start=True, stop=not edge)
            if edge:
                for h in range(6):
                    nc.tensor.matmul(
                        PS[:, 512 * h:512 * h + SLAB],
                        lhsT=ones1[0:1, :],
                        rhs=kmask[0:1, k0:k0 + SLAB],
                        start=False, stop=True)

        def emit_exp(b):
            E = sbuf.tile([128, 6, SLAB], f16, tag="E")
            tiles["E", b] = E
            nc.scalar.activation(
                out=E[:, :, :],
                in_=apx(PS[:, :], [[512, 6], [1, SLAB]]),
                func=AF.Exp)

        def emit_chain(b):
            E = tiles["E", b]
            P = sbuf.tile([128, 6, SLAB], f16, tag="P")
            sums = sbuf.tile([128, 6], f32, tag="sums")
            nc.vector.tensor_mul(P[:, :, :], E[:, :, :], band[:, :, :])
            nc.vector.reduce_sum(out=sums, in_=P[:, :, :], axis=AX.X)
            recip = sbuf.tile([128, 6], f32, tag="recip")
            nc.vector.reciprocal(recip, sums)
            Ph = Phb[b % 2]
            for h in range(6):
                nc.vector.tensor_scalar_mul(Ph[:, h, 0:SLAB], P[:, h, :],
                                            recip[:, h:h + 1])
            tiles["Ph", b] = Ph

        def emit_trans(b):
            Ph = tiles["Ph", b]
            PTa = sbuf.tile([128, 6, 128], f16, tag="PTa")
            Tb = sbuf.tile([16, 6, 128], f16, tag="Tb")
            tiles["PTa", b] = PTa
            tiles["Tb", b] = Tb
            for h in range(6):
                nc.tensor.matmul(
                    ps16[:, 1024 * h:1024 * h + 128],
                    lhsT=Ph[:, h, 0:128], rhs=ident,
                    is_transpose=True, start=True, stop=True)
            nc.vector.tensor_copy(PTa[:, :, :],
                                  apx(ps16[:, 0:128], [[1024, 6], [1, 128]]))
            for wv in range(3):
                for t in range(2):
                    nc.tensor.matmul(
                        ps16[0:W2, 1024 * (6 + t):1024 * (6 + t) + 128],
                        lhsT=Ph[:, 2 * wv + t, 128:128 + W2], rhs=ident,
                        is_transpose=True, start=True, stop=True)
                nc.vector.tensor_copy(
                    Tb[:, 2 * wv:2 * wv + 2, :],
                    apx(ps16[0:16, 0:128], [[1024, 2], [1, 128]],
                        extra_off=6 * 1024))

        def emit_pv_proj(b):
            PTa = tiles.pop(("PTa", b))
            Tb = tiles.pop(("Tb", b))
            tiles.pop(("E", b), None)
            tiles.pop(("Ph", b), None)
            aT = sbuf.tile([128, 3, 128], f16, tag="aT")
            for j in range(3):
                coff = 512 * (6 + (j & 1))
                h0, h1 = 2 * j, 2 * j + 1
                mm0 = nc.tensor.matmul(
                    PS[0:64, coff:coff + 128],
                    lhsT=V[:, b, 64 * h0:64 * h0 + 64],
                    rhs=PTa[:, h0, :], start=True, stop=False)
                mm1 = nc.tensor.matmul(
                    PS[0:64, coff:coff + 128],
                    lhsT=V[0:W2, b + 1, 64 * h0:64 * h0 + 64],
                    rhs=Tb[0:W2, h0, :], start=False, stop=True)
                mm2 = nc.tensor.matmul(
                    PS[64:128, coff:coff + 128],
                    lhsT=V[:, b, 64 * h1:64 * h1 + 64],
                    rhs=PTa[:, h1, :], start=True, stop=False)
                mm3 = nc.tensor.matmul(
                    PS[64:128, coff:coff + 128],
                    lhsT=V[0:W2, b + 1, 64 * h1:64 * h1 + 64],
                    rhs=Tb[0:W2, h1, :], start=False, stop=True)
                add_dep_helper(mm1.ins, mm0.ins, sync=False, reason="grp order")
                add_dep_helper(mm2.ins, mm1.ins, sync=False, reason="grp order")
                add_dep_helper(mm3.ins, mm2.ins, sync=False, reason="grp order")
                if j == 1:
                    nc.vector.tensor_copy(aT[:, j, :], PS[:, coff:coff + 128])
                else:
                    nc.scalar.copy(aT[:, j, :], PS[:, coff:coff + 128])
            yoff = 512 * 7
            for jj in range(3):
                nc.tensor.matmul(
                    PS[:, yoff:yoff + C],
                    lhsT=aT[:, jj, :], rhs=pw[:, jj, :],
                    start=(jj == 0), stop=(jj == 2 and not with_bias))
            if with_bias:
                nc.tensor.matmul(PS[:, yoff:yoff + C],
                                 lhsT=ones1[0:1, :], rhs=pb[0:1, :],
                                 start=False, stop=True)
            Yf = sbuf.tile([128, C], f16, tag="Yf")
            nc.vector.tensor_copy(Yf, PS[:, yoff:yoff + C])
            nc.sync.dma_start(out=out_d[128 * b:128 * (b + 1), :], in_=Yf)

        # 2-stage software pipeline: block b's scores/softmax run while
        # block b-1's transposes/PV/projection drain on banks 6,7
        for b in range(NBLK):
            if b >= 1:
                emit_trans(b - 1)
            emit_scores(b)
            emit_exp(b)
            if b >= 1:
                emit_pv_proj(b - 1)
            emit_chain(b)
        emit_trans(NBLK - 1)
        emit_pv_proj(NBLK - 1)

    nc.finalize()
    return nc


def make_in_maps(x, qkv_w, proj_w, proj_b, w, with_bias):
    W2 = 2 * w
    RT = R + W2
    SLAB = 128 + W2

    qk_w = qkv_w[:2 * C].copy()
    qk_w[:C] *= SCALE                       # fold softmax scale into Q weights
    qk_wT = np.ascontiguousarray(qk_w.T).astype(np.float16)
    v_wT = np.ascontiguousarray(qkv_w[2 * C:].T).astype(np.float16)
    proj_wT = np.ascontiguousarray(proj_w.T).astype(np.float16)
    pb16 = proj_b.reshape(1, C).astype(np.float16)
    ident = np.eye(128, dtype=np.float16)

    p = np.arange(128)[:, None]
    t = np.arange(SLAB)[None, :]
    band = ((t >= p) & (t <= p + W2)).astype(np.float16)
    band = np.tile(band, (1, 6))

    in_maps = []
    for c in range(NCORES):
        b, s = divmod(c, SEQ_SHARDS)
        g0 = s * R
        xpad = np.zeros((RT, C), dtype=np.float32)
        lo = max(0, g0 - w)
        hi = min(N, g0 + R + w)
        xpad[lo - (g0 - w): hi - (g0 - w)] = x[b, lo:hi]
        xT = np.ascontiguousarray(xpad.T).astype(np.float16)
        kmask = np.zeros((1, RT), dtype=np.float16)
        if s == 0:
            kmask[0, :w] = NEG
        if s == SEQ_SHARDS - 1:
            kmask[0, R + w:] = NEG
        m = {"xT": xT, "qk_wT": qk_wT, "v_wT": v_wT, "proj_wT": proj_wT,
             "band": band, "kmask": kmask, "ident": ident}
        if with_bias:
            m["proj_b16"] = pb16
        in_maps.append(m)
    return in_maps


def kernel(x, qkv_w, proj_w, proj_b, epoch):
    x = np.asarray(x, dtype=np.float32)
    qkv_w = np.asarray(qkv_w, dtype=np.float32)
    proj_w = np.asarray(proj_w, dtype=np.float32)
    proj_b = np.asarray(proj_b, dtype=np.float32)
    w = _window_for_epoch(int(np.asarray(epoch)))
    if w is None or 128 + 2 * w > 512:
        return _numpy_fallback(x, qkv_w, proj_w, proj_b, w)

    from concourse.bass_utils import run_bass_kernel_spmd

    with_bias = bool(np.any(proj_b != 0.0))
    key = (w, with_bias)
    if key not in _PROGRAM_CACHE:
        _PROGRAM_CACHE[key] = build_program(w, with_bias)
    nc = _PROGRAM_CACHE[key]

    in_maps = make_in_maps(x, qkv_w, proj_w, proj_b, w, with_bias)
    res = run_bass_kernel_spmd(nc, in_maps, core_ids=list(range(NCORES)))

    out = np.empty((B, N, C), dtype=np.float32)
    for c in range(NCORES):
        b, s = divmod(c, SEQ_SHARDS)
        out[b, s * R:(s + 1) * R] = res.results[c]["out"].astype(np.float32)
    return out


# revision 38
# speedup vs baseline: 1.1546x; 1.1546x over previous
"""Banded (sliding-window) attention kernel for Trainium2, 8 NeuronCores.

Problem: nn_AttentionLAI (B=2, N=4096, C=384, H=6, head_dim=64), epoch=0
=> band window w=8 (each query attends keys with |i-j| <= 8).

Sharding: sequence sharding. core c = b*4 + s handles batch b, rows
[s*1024, (s+1)*1024) with a w-row halo on each side.  Zero collectives;
host gathers the 8 per-core outputs.

PSUM discipline (empirically validated on HW): at most ONE matmul output
column-range per 2KB bank at a time; writes at different partition
offsets to that same column range are fine; sequential reuse of a bank
with a different range is fine once prior readers complete.  The whole
PSUM is managed as one [128, 4096] f32 tile with manual bank regions.

Per-core dataflow (fp16 operands -> f32 PSUM accumulate):
  xT   [384, RT]   x-shard transposed (feature-major), RT = 1024 + 2w
  qkT  [768, RT]   = qk_w^T-chunks @ xT     (feature-major Q^T / K^T)
  V    [RT, 384]   = xT-chunks^T @ v_w^T    (row-major V)
  per 128-query block b (8 blocks):
    S_h  = Q^T_h' K^T_hslab -> PSUM bank h            (+ edge kmask acc)
    E    = exp(S)  (one ACT op, strided across banks)
    P    = E * band_mask;  sums = rowsum(P);  Phat = P / sums  (DVE)
    PT   = transpose(Phat) -> banks 0-5 main, 6-7 + 0-3 tails; bounce
    O^T  = V^T-chunks: 12 matmuls -> banks 4-6 (2 heads/bank partition-
           packed); bounce -> aT (feature-major, ready for projection)
    Y    = aT^T @ proj_w^T (+bias) -> bank 7; bounce; DMA out rows.
"""

import numpy as np

B, N, C, H = 2, 4096, 384, 6
HD = C // H            # 64
SCALE = HD ** -0.5
MASK_EPOCHS = [10, 20, 30, 40]
NCORES = 8
SEQ_SHARDS = 4         # per batch
R = N // SEQ_SHARDS    # 1024 rows per core
NBLK = R // 128        # 8 query blocks per core
NEG = -60000.0         # additive mask value (fits fp16, exp() -> 0)


def _window_for_epoch(epoch):
    if epoch >= MASK_EPOCHS[-1]:
        return None
    if epoch < MASK_EPOCHS[-4]:
        return 8
    elif epoch < MASK_EPOCHS[-3]:
        return 12
    elif epoch < MASK_EPOCHS[-2]:
        return 18
    else:
        return 20


def _numpy_fallback(x, qkv_w, proj_w, proj_b, w):
    """Reference-equivalent host computation (used only for epoch>=40)."""
    b, n, c = x.shape
    qkv = (x @ qkv_w.T).reshape(b, n, 3, H, HD).transpose(2, 0, 3, 1, 4)
    q, k, v = qkv[0], qkv[1], qkv[2]
    attn = np.einsum("bhnd,bhmd->bhnm", q, k) * SCALE
    if w is not None:
        idx = np.arange(n)
        band = np.abs(idx[:, None] - idx[None, :]) <= w
        attn = np.where(band[None, None], attn, -1e9)
    attn = attn - attn.max(axis=-1, keepdims=True)
    attn = np.exp(attn)
    attn = attn / attn.sum(axis=-1, keepdims=True)
    out = np.einsum("bhnm,bhmd->bhnd", attn, v)
    out = out.transpose(0, 2, 1, 3).reshape(b, n, c)
    return (out @ proj_w.T + proj_b).astype(np.float32)


_PROGRAM_CACHE = {}


def build_program(w, with_bias):
    """Build the SPMD Bass program for band window w. Returns nc."""
    import concourse.bass as bass
    import concourse.mybir as mybir
    import concourse.tile as tile
    from concourse import bacc
    from concourse.tile import add_dep_helper
    from contextlib import ExitStack

    f16 = mybir.dt.float16
    f32 = mybir.dt.float32
    AF = mybir.ActivationFunctionType
    AX = mybir.AxisListType

    W2 = 2 * w
    SLAB = 128 + W2            # keys per 128-query block
    RT = R + W2                # haloed rows per core
    assert SLAB <= 512
    NCH = [(0, 512), (512, 512), (1024, RT - 1024)]  # qkT col chunks
    NVCH = RT // 128 + (1 if RT % 128 else 0)        # V row chunks (9)

    nc = bacc.Bacc()

    xT_d = nc.declare_dram_parameter("xT", [C, RT], f16, isOutput=False)
    qkw_d = nc.declare_dram_parameter("qk_wT", [C, 2 * C], f16, isOutput=False)
    vw_d = nc.declare_dram_parameter("v_wT", [C, C], f16, isOutput=False)
    pw_d = nc.declare_dram_parameter("proj_wT", [C, C], f16, isOutput=False)
    band_d = nc.declare_dram_parameter("band", [128, 6 * SLAB], f16, isOutput=False)
    kmask_d = nc.declare_dram_parameter("kmask", [1, RT], f16, isOutput=False)
    ident_d = nc.declare_dram_parameter("ident", [128, 128], f16, isOutput=False)
    if with_bias:
        pb_d = nc.declare_dram_parameter("proj_b16", [1, C], f16, isOutput=False)
    out_d = nc.declare_dram_parameter("out", [R, C], f16, isOutput=True)

    def apx(sl, dims, extra_off=0):
        """AP with custom free dims on top of a slice's partition dim."""
        return bass.AP(tensor=sl.tensor, offset=sl.offset + extra_off,
                       ap=[sl.ap[0]] + dims)

    with ExitStack() as ctx:
        tc = ctx.enter_context(tile.TileContext(nc))
        consts = ctx.enter_context(tc.tile_pool(name="consts", bufs=1))

        xT = consts.tile([128, 3, RT], f16)
        qkw = consts.tile([128, 3, 2 * C], f16)
        vw = consts.tile([128, 3, C], f16)
        pw = consts.tile([128, 3, C], f16)
        band = consts.tile([128, 6, SLAB], f16)
        kmask = consts.tile([1, RT], f16)
        ident = consts.tile([128, 128], f16)
        ones1 = consts.tile([1, 128], f16)
        nc.vector.memset(ones1, 1.0)
        if with_bias:
            pb = consts.tile([1, C], f16)
            nc.sync.dma_start(out=pb, in_=pb_d[:, :])
        # spread input DMAs over four queues; phase-1's first deps come first
        half = RT // 2
        for j in range(3):
            nc.sync.dma_start(out=xT[:, j, 0:half],
                              in_=xT_d[128 * j:128 * (j + 1), 0:half])
            nc.scalar.dma_start(out=xT[:, j, half:RT],
                                in_=xT_d[128 * j:128 * (j + 1), half:RT])
            nc.gpsimd.dma_start(out=qkw[:, j, :], in_=qkw_d[128 * j:128 * (j + 1), :])
        for j in range(3):
            nc.gpsimd.dma_start(out=vw[:, j, :], in_=vw_d[128 * j:128 * (j + 1), :])
            nc.scalar.dma_start(out=pw[:, j, :], in_=pw_d[128 * j:128 * (j + 1), :])
        nc.scalar.dma_start(out=band[:, :, :], in_=band_d[:, :])
        nc.gpsimd.dma_start(out=kmask, in_=kmask_d[:, :])
        nc.gpsimd.dma_start(out=ident, in_=ident_d[:, :])

        qkT = consts.tile([128, 6, RT], f16)
        # per-head Q with the other head's 64 rows zeroed: lets the score
        # matmuls run at contract-128 / base-0 (walrus splits contract-64
        # matmuls into two HW instructions)
        qkQ = consts.tile([128, 6, RT], f16)
        for par in range(2):
            zsl = qkQ[64 * (1 - par):64 * (2 - par), par, :]
            nc.vector.memset(
                bass.AP(tensor=zsl.tensor, offset=zsl.offset,
                        ap=[zsl.ap[0], [2 * RT, 3], [1, RT]]), 0.0)
        V = consts.tile([128, NVCH, C], f16)
        # ping/pong normalized-score buffers; slot padded to 256 so the
        # 16-wide tail can be DMA-transposed as a full 128-col window
        Phb = []
        for i in range(2):
            Phx = consts.tile([128, 6, 256], f16, tag="Phb%d" % i)
            nc.gpsimd.memset(Phx[:, :, SLAB:256], 0.0)
            Phb.append(Phx)

        # ---- phase 1: qkT = qk_w^T @ x^T  (feature-major) ----
        eng = 0
        with ExitStack() as ph1:
            qkps = ph1.enter_context(tc.tile_pool(name="qkps", bufs=6, space="PSUM"))
            for (n0, ns) in NCH:
                for j in range(6):
                    ps = qkps.tile([128, 512], f32, tag="qkps")
                    for cc in range(3):
                        nc.tensor.matmul(
                            ps[:, :ns],
                            lhsT=qkw[:, cc, 128 * j:128 * (j + 1)],
                            rhs=xT[:, cc, n0:n0 + ns],
                            start=(cc == 0), stop=(cc == 2))
                    if j < 3:
                        nc.vector.tensor_copy(
                            qkQ[0:64, 2 * j, n0:n0 + ns], ps[0:64, :ns])
                        nc.scalar.copy(
                            qkQ[64:128, 2 * j + 1, n0:n0 + ns], ps[64:128, :ns])
                    elif eng % 2 == 0:
                        nc.vector.tensor_copy(qkT[:, j, n0:n0 + ns], ps[:, :ns])
                    else:
                        nc.scalar.copy(qkT[:, j, n0:n0 + ns], ps[:, :ns])
                    eng += 1
            # ---- phase 2: V = (x^T)^T @ v_w^T  (row-major) ----
            for r in range(NVCH):
                rlen = min(128, RT - 128 * r)
                ps = qkps.tile([128, 512], f32, tag="qkps")
                for cc in range(3):
                    nc.tensor.matmul(
                        ps[:rlen, :C],
                        lhsT=xT[:, cc, 128 * r:128 * r + rlen],
                        rhs=vw[:, cc, :],
                        start=(cc == 0), stop=(cc == 2))
                if eng % 2 == 0:
                    nc.vector.tensor_copy(V[:rlen, r, :], ps[:rlen, :C])
                else:
                    nc.scalar.copy(V[:rlen, r, :], ps[:rlen, :C])
                eng += 1


        # ---- phase 3: per-block attention + projection ----
        pspool = ctx.enter_context(tc.tile_pool(name="ps8", bufs=1, space="PSUM"))
        PS = pspool.tile([128, 4096], f32)          # the whole PSUM
        ps16 = PS[:, :].bitcast(f16)                # [128, 8192] fp16 view
        sbuf = ctx.enter_context(tc.tile_pool(name="work", bufs=2))

        tiles = {}

        def emit_scores(b):
            q0 = w + 128 * b
            k0 = 128 * b
            edge = b in (0, NBLK - 1)
            for h in range(6):
                nc.tensor.matmul(
                    PS[:, 512 * h:512 * h + SLAB],
                    lhsT=qkQ[:, h, q0:q0 + 128],
                    rhs=qkT[:, 3 + h // 2, k0:k0 + SLAB],
                    start=True, stop=not edge)
            if edge:
                for h in range(6):
                    nc.tensor.matmul(
                        PS[:, 512 * h:512 * h + SLAB],
                        lhsT=ones1[0:1, :],
                        rhs=kmask[0:1, k0:k0 + SLAB],
                        start=False, stop=True)

        def emit_exp(b):
            E = sbuf.tile([128, 6, SLAB], f16, tag="E")
            tiles["E", b] = E
            nc.scalar.activation(
                out=E[:, :, :],
                in_=apx(PS[:, :], [[512, 6], [1, SLAB]]),
                func=AF.Exp)

        def emit_chain(b):
            E = tiles["E", b]
            P = sbuf.tile([128, 6, SLAB], f16, tag="P")
            sums = sbuf.tile([128, 6], f32, tag="sums")
            nc.vector.tensor_mul(P[:, :, :], E[:, :, :], band[:, :, :])
            nc.vector.reduce_sum(out=sums, in_=P[:, :, :], axis=AX.X)
            recip = sbuf.tile([128, 6], f32, tag="recip")
            nc.vector.reciprocal(recip, sums)
            Ph = Phb[b % 2]
            for h in range(6):
                nc.vector.tensor_scalar_mul(Ph[:, h, 0:SLAB], P[:, h, :],
                                            recip[:, h:h + 1])
            tiles["Ph", b] = Ph

        def emit_trans(b):
            Ph = tiles["Ph", b]
            PTa = sbuf.tile([128, 6, 128], f16, tag="PTa")
            Tb = sbuf.tile([16, 6, 128], f16, tag="Tb")
            tiles["PTa", b] = PTa
            tiles["Tb", b] = Tb
            for h in range(6):
                nc.tensor.matmul(
                    ps16[:, 1024 * h:1024 * h + 128],
                    lhsT=Ph[:, h, 0:128], rhs=ident,
                    is_transpose=True, start=True, stop=True)
            nc.vector.tensor_copy(PTa[:, :, :],
                                  apx(ps16[:, 0:128], [[1024, 6], [1, 128]]))
            for wv in range(3):
                for t in range(2):
                    nc.tensor.matmul(
                        ps16[0:W2, 1024 * (6 + t):1024 * (6 + t) + 128],
                        lhsT=Ph[:, 2 * wv + t, 128:128 + W2], rhs=ident,
                        is_transpose=True, start=True, stop=True)
                nc.vector.tensor_copy(
                    Tb[:, 2 * wv:2 * wv + 2, :],
                    apx(ps16[0:16, 0:128], [[1024, 2], [1, 128]],
                        extra_off=6 * 1024))

        def emit_pv_proj(b):
            PTa = tiles.pop(("PTa", b))
            Tb = tiles.pop(("Tb", b))
            tiles.pop(("E", b), None)
            tiles.pop(("Ph", b), None)
            aT = sbuf.tile([128, 3, 128], f16, tag="aT")
            for j in range(3):
                coff = 512 * (6 + (j & 1))
                h0, h1 = 2 * j, 2 * j + 1
                mm0 = nc.tensor.matmul(
                    PS[0:64, coff:coff + 128],
                    lhsT=V[:, b, 64 * h0:64 * h0 + 64],
                    rhs=PTa[:, h0, :], start=True, stop=False)
                mm1 = nc.tensor.matmul(
                    PS[0:64, coff:coff + 128],
                    lhsT=V[0:W2, b + 1, 64 * h0:64 * h0 + 64],
                    rhs=Tb[0:W2, h0, :], start=False, stop=True)
                mm2 = nc.tensor.matmul(
                    PS[64:128, coff:coff + 128],
                    lhsT=V[:, b, 64 * h1:64 * h1 + 64],
                    rhs=PTa[:, h1, :], start=True, stop=False)
                mm3 = nc.tensor.matmul(
                    PS[64:128, coff:coff + 128],
                    lhsT=V[0:W2, b + 1, 64 * h1:64 * h1 + 64],
                    rhs=Tb[0:W2, h1, :], start=False, stop=True)
                add_dep_helper(mm1.ins, mm0.ins, sync=False, reason="grp order")
                add_dep_helper(mm2.ins, mm1.ins, sync=False, reason="grp order")
                add_dep_helper(mm3.ins, mm2.ins, sync=False, reason="grp order")
                if j == 1:
                    nc.vector.tensor_copy(aT[:, j, :], PS[:, coff:coff + 128])
                else:
                    nc.scalar.copy(aT[:, j, :], PS[:, coff:coff + 128])
            yoff = 512 * 7
            for jj in range(3):
                nc.tensor.matmul(
                    PS[:, yoff:yoff + C],
                    lhsT=aT[:, jj, :], rhs=pw[:, jj, :],
                    start=(jj == 0), stop=(jj == 2 and not with_bias))
            if with_bias:
                nc.tensor.matmul(PS[:, yoff:yoff + C],
                                 lhsT=ones1[0:1, :], rhs=pb[0:1, :],
                                 start=False, stop=True)
            Yf = sbuf.tile([128, C], f16, tag="Yf")
            nc.vector.tensor_copy(Yf, PS[:, yoff:yoff + C])
            nc.sync.dma_start(out=out_d[128 * b:128 * (b + 1), :], in_=Yf)

        # 2-stage software pipeline: block b's scores/softmax run while
        # block b-1's transposes/PV/projection drain on banks 6,7
        for b in range(NBLK):
            if b >= 1:
                emit_trans(b - 1)
            emit_scores(b)
            emit_exp(b)
            if b >= 1:
                emit_pv_proj(b - 1)
            emit_chain(b)
        emit_trans(NBLK - 1)
        emit_pv_proj(NBLK - 1)

    nc.finalize()
    return nc


def make_in_maps(x, qkv_w, proj_w, proj_b, w, with_bias):
    W2 = 2 * w
    RT = R + W2
    SLAB = 128 + W2

    qk_w = qkv_w[:2 * C].copy()
    qk_w[:C] *= SCALE                       # fold softmax scale into Q weights
    qk_wT = np.ascontiguousarray(qk_w.T).astype(np.float16)
    v_wT = np.ascontiguousarray(qkv_w[2 * C:].T).astype(np.float16)
    proj_wT = np.ascontiguousarray(proj_w.T).astype(np.float16)
    pb16 = proj_b.reshape(1, C).astype(np.float16)
    ident = np.eye(128, dtype=np.float16)

    p = np.arange(128)[:, None]
    t = np.arange(SLAB)[None, :]
    band = ((t >= p) & (t <= p + W2)).astype(np.float16)
    band = np.tile(band, (1, 6))

    in_maps = []
    for c in range(NCORES):
        b, s = divmod(c, SEQ_SHARDS)
        g0 = s * R
        xpad = np.zeros((RT, C), dtype=np.float32)
        lo = max(0, g0 - w)
        hi = min(N, g0 + R + w)
        xpad[lo - (g0 - w): hi - (g0 - w)] = x[b, lo:hi]
        xT = np.ascontiguousarray(xpad.T).astype(np.float16)
        kmask = np.zeros((1, RT), dtype=np.float16)
        if s == 0:
            kmask[0, :w] = NEG
        if s == SEQ_SHARDS - 1:
            kmask[0, R + w:] = NEG
        m = {"xT": xT, "qk_wT": qk_wT, "v_wT": v_wT, "proj_wT": proj_wT,
             "band": band, "kmask": kmask, "ident": ident}
        if with_bias:
            m["proj_b16"] = pb16
        in_maps.append(m)
    return in_maps


def kernel(x, qkv_w, proj_w, proj_b, epoch):
    x = np.asarray(x, dtype=np.float32)
    qkv_w = np.asarray(qkv_w, dtype=np.float32)
    proj_w = np.asarray(proj_w, dtype=np.float32)
    proj_b = np.asarray(proj_b, dtype=np.float32)
    w = _window_for_epoch(int(np.asarray(epoch)))
    if w is None or 128 + 2 * w > 512:
        return _numpy_fallback(x, qkv_w, proj_w, proj_b, w)

    from concourse.bass_utils import run_bass_kernel_spmd

    with_bias = bool(np.any(proj_b != 0.0))
    key = (w, with_bias)
    if key not in _PROGRAM_CACHE:
        _PROGRAM_CACHE[key] = build_program(w, with_bias)
    nc = _PROGRAM_CACHE[key]

    in_maps = make_in_maps(x, qkv_w, proj_w, proj_b, w, with_bias)
    res = run_bass_kernel_spmd(nc, in_maps, core_ids=list(range(NCORES)))

    out = np.empty((B, N, C), dtype=np.float32)
    for c in range(NCORES):
        b, s = divmod(c, SEQ_SHARDS)
        out[b, s * R:(s + 1) * R] = res.results[c]["out"].astype(np.float32)
    return out


# revision 39
# speedup vs baseline: 1.1898x; 1.0305x over previous
"""Banded (sliding-window) attention kernel for Trainium2, 8 NeuronCores.

Problem: nn_AttentionLAI (B=2, N=4096, C=384, H=6, head_dim=64), epoch=0
=> band window w=8 (each query attends keys with |i-j| <= 8).

Sharding: sequence sharding. core c = b*4 + s handles batch b, rows
[s*1024, (s+1)*1024) with a w-row halo on each side.  Zero collectives;
host gathers the 8 per-core outputs.

PSUM discipline (empirically validated on HW): at most ONE matmul output
column-range per 2KB bank at a time; writes at different partition
offsets to that same column range are fine; sequential reuse of a bank
with a different range is fine once prior readers complete.  The whole
PSUM is managed as one [128, 4096] f32 tile with manual bank regions.

Per-core dataflow (fp16 operands -> f32 PSUM accumulate):
  xT   [384, RT]   x-shard transposed (feature-major), RT = 1024 + 2w
  qkT  [768, RT]   = qk_w^T-chunks @ xT     (feature-major Q^T / K^T)
  V    [RT, 384]   = xT-chunks^T @ v_w^T    (row-major V)
  per 128-query block b (8 blocks):
    S_h  = Q^T_h' K^T_hslab -> PSUM bank h            (+ edge kmask acc)
    E    = exp(S)  (one ACT op, strided across banks)
    P    = E * band_mask;  sums = rowsum(P);  Phat = P / sums  (DVE)
    PT   = transpose(Phat) -> banks 0-5 main, 6-7 + 0-3 tails; bounce
    O^T  = V^T-chunks: 12 matmuls -> banks 4-6 (2 heads/bank partition-
           packed); bounce -> aT (feature-major, ready for projection)
    Y    = aT^T @ proj_w^T (+bias) -> bank 7; bounce; DMA out rows.
"""

import numpy as np

B, N, C, H = 2, 4096, 384, 6
HD = C // H            # 64
SCALE = HD ** -0.5
MASK_EPOCHS = [10, 20, 30, 40]
NCORES = 8
SEQ_SHARDS = 4         # per batch
R = N // SEQ_SHARDS    # 1024 rows per core
NBLK = R // 128        # 8 query blocks per core
NEG = -60000.0         # additive mask value (fits fp16, exp() -> 0)


def _window_for_epoch(epoch):
    if epoch >= MASK_EPOCHS[-1]:
        return None
    if epoch < MASK_EPOCHS[-4]:
        return 8
    elif epoch < MASK_EPOCHS[-3]:
        return 12
    elif epoch < MASK_EPOCHS[-2]:
        return 18
    else:
        return 20


def _numpy_fallback(x, qkv_w, proj_w, proj_b, w):
    """Reference-equivalent host computation (used only for epoch>=40)."""
    b, n, c = x.shape
    qkv = (x @ qkv_w.T).reshape(b, n, 3, H, HD).transpose(2, 0, 3, 1, 4)
    q, k, v = qkv[0], qkv[1], qkv[2]
    attn = np.einsum("bhnd,bhmd->bhnm", q, k) * SCALE
    if w is not None:
        idx = np.arange(n)
        band = np.abs(idx[:, None] - idx[None, :]) <= w
        attn = np.where(band[None, None], attn, -1e9)
    attn = attn - attn.max(axis=-1, keepdims=True)
    attn = np.exp(attn)
    attn = attn / attn.sum(axis=-1, keepdims=True)
    out = np.einsum("bhnm,bhmd->bhnd", attn, v)
    out = out.transpose(0, 2, 1, 3).reshape(b, n, c)
    return (out @ proj_w.T + proj_b).astype(np.float32)


_PROGRAM_CACHE = {}


def build_program(w, with_bias):
    """Build the SPMD Bass program for band window w. Returns nc."""
    import concourse.bass as bass
    import concourse.mybir as mybir
    import concourse.tile as tile
    from concourse import bacc
    from concourse.tile import add_dep_helper
    from contextlib import ExitStack

    f16 = mybir.dt.float16
    f32 = mybir.dt.float32
    AF = mybir.ActivationFunctionType
    AX = mybir.AxisListType

    W2 = 2 * w
    SLAB = 128 + W2            # keys per 128-query block
    RT = R + W2                # haloed rows per core
    assert SLAB <= 512
    NCH = [(0, 512), (512, 512), (1024, RT - 1024)]  # qkT col chunks
    NVCH = RT // 128 + (1 if RT % 128 else 0)        # V row chunks (9)

    nc = bacc.Bacc()

    xT_d = nc.declare_dram_parameter("xT", [C, RT], f16, isOutput=False)
    qkw_d = nc.declare_dram_parameter("qk_wT", [C, 2 * C], f16, isOutput=False)
    vw_d = nc.declare_dram_parameter("v_wT", [C, C], f16, isOutput=False)
    pw_d = nc.declare_dram_parameter("proj_wT", [C, C], f16, isOutput=False)
    band_d = nc.declare_dram_parameter("band", [128, 6 * SLAB], f16, isOutput=False)
    kmask_d = nc.declare_dram_parameter("kmask", [1, RT], f16, isOutput=False)
    ident_d = nc.declare_dram_parameter("ident", [128, 128], f16, isOutput=False)
    if with_bias:
        pb_d = nc.declare_dram_parameter("proj_b16", [1, C], f16, isOutput=False)
    out_d = nc.declare_dram_parameter("out", [R, C], f16, isOutput=True)

    def apx(sl, dims, extra_off=0):
        """AP with custom free dims on top of a slice's partition dim."""
        return bass.AP(tensor=sl.tensor, offset=sl.offset + extra_off,
                       ap=[sl.ap[0]] + dims)

    with ExitStack() as ctx:
        tc = ctx.enter_context(tile.TileContext(nc))
        consts = ctx.enter_context(tc.tile_pool(name="consts", bufs=1))

        xT = consts.tile([128, 3, RT], f16)
        qkw = consts.tile([128, 3, 2 * C], f16)
        vw = consts.tile([128, 3, C], f16)
        pw = consts.tile([128, 3, C], f16)
        band = consts.tile([128, 6, SLAB], f16)
        kmask = consts.tile([1, RT], f16)
        ident = consts.tile([128, 128], f16)
        ones1 = consts.tile([1, 128], f16)
        nc.vector.memset(ones1, 1.0)
        if with_bias:
            pb = consts.tile([1, C], f16)
            nc.sync.dma_start(out=pb, in_=pb_d[:, :])
        # spread input DMAs over four queues; phase-1's first deps come first
        half = RT // 2
        for j in range(3):
            nc.sync.dma_start(out=xT[:, j, 0:half],
                              in_=xT_d[128 * j:128 * (j + 1), 0:half])
            nc.scalar.dma_start(out=xT[:, j, half:RT],
                                in_=xT_d[128 * j:128 * (j + 1), half:RT])
            nc.gpsimd.dma_start(out=qkw[:, j, :], in_=qkw_d[128 * j:128 * (j + 1), :])
        for j in range(3):
            nc.gpsimd.dma_start(out=vw[:, j, :], in_=vw_d[128 * j:128 * (j + 1), :])
            nc.scalar.dma_start(out=pw[:, j, :], in_=pw_d[128 * j:128 * (j + 1), :])
        nc.scalar.dma_start(out=band[:, :, :], in_=band_d[:, :])
        nc.gpsimd.dma_start(out=kmask, in_=kmask_d[:, :])
        nc.gpsimd.dma_start(out=ident, in_=ident_d[:, :])

        qkT = consts.tile([128, 6, RT], f16)
        # per-head Q with the other head's 64 rows zeroed: lets the score
        # matmuls run at contract-128 / base-0 (walrus splits contract-64
        # matmuls into two HW instructions)
        qkQ = consts.tile([128, 6, RT], f16)
        for par in range(2):
            zsl = qkQ[64 * (1 - par):64 * (2 - par), par, :]
            nc.vector.memset(
                bass.AP(tensor=zsl.tensor, offset=zsl.offset,
                        ap=[zsl.ap[0], [2 * RT, 3], [1, RT]]), 0.0)
        V = consts.tile([128, NVCH, C], f16)
        # ping/pong normalized-score buffers; slot padded to 256 so the
        # 16-wide tail can be DMA-transposed as a full 128-col window
        Phb = []
        for i in range(2):
            Phx = consts.tile([128, 6, 256], f16, tag="Phb%d" % i)
            nc.gpsimd.memset(Phx[:, :, SLAB:256], 0.0)
            Phb.append(Phx)

        # ---- phase 1: qkT = qk_w^T @ x^T  (feature-major) ----
        eng = 0
        with ExitStack() as ph1:
            qkps = ph1.enter_context(tc.tile_pool(name="qkps", bufs=6, space="PSUM"))
            for (n0, ns) in NCH:
                for j in range(6):
                    ps = qkps.tile([128, 512], f32, tag="qkps")
                    for cc in range(3):
                        nc.tensor.matmul(
                            ps[:, :ns],
                            lhsT=qkw[:, cc, 128 * j:128 * (j + 1)],
                            rhs=xT[:, cc, n0:n0 + ns],
                            start=(cc == 0), stop=(cc == 2))
                    if j < 3:
                        nc.vector.tensor_copy(
                            qkQ[0:64, 2 * j, n0:n0 + ns], ps[0:64, :ns])
                        nc.scalar.copy(
                            qkQ[64:128, 2 * j + 1, n0:n0 + ns], ps[64:128, :ns])
                    elif eng % 2 == 0:
                        nc.vector.tensor_copy(qkT[:, j, n0:n0 + ns], ps[:, :ns])
                    else:
                        nc.scalar.copy(qkT[:, j, n0:n0 + ns], ps[:, :ns])
                    eng += 1


        # ---- phase 3: per-block attention + projection ----
        pspool = ctx.enter_context(tc.tile_pool(name="ps8", bufs=1, space="PSUM"))
        PS = pspool.tile([128, 4096], f32)          # the whole PSUM
        ps16 = PS[:, :].bitcast(f16)                # [128, 8192] fp16 view
        sbuf = ctx.enter_context(tc.tile_pool(name="work", bufs=2))

        tiles = {}

        def emit_scores(b):
            q0 = w + 128 * b
            k0 = 128 * b
            edge = b in (0, NBLK - 1)
            for h in range(6):
                nc.tensor.matmul(
                    PS[:, 512 * h:512 * h + SLAB],
                    lhsT=qkQ[:, h, q0:q0 + 128],
                    rhs=qkT[:, 3 + h // 2, k0:k0 + SLAB],
                    start=True, stop=not edge)
            if edge:
                for h in range(6):
                    nc.tensor.matmul(
                        PS[:, 512 * h:512 * h + SLAB],
                        lhsT=ones1[0:1, :],
                        rhs=kmask[0:1, k0:k0 + SLAB],
                        start=False, stop=True)

        def emit_exp(b):
            E = sbuf.tile([128, 6, SLAB], f16, tag="E")
            tiles["E", b] = E
            nc.scalar.activation(
                out=E[:, :, :],
                in_=apx(PS[:, :], [[512, 6], [1, SLAB]]),
                func=AF.Exp)

        def emit_chain(b):
            E = tiles["E", b]
            P = sbuf.tile([128, 6, SLAB], f16, tag="P")
            sums = sbuf.tile([128, 6], f32, tag="sums")
            nc.vector.tensor_mul(P[:, :, :], E[:, :, :], band[:, :, :])
            nc.vector.reduce_sum(out=sums, in_=P[:, :, :], axis=AX.X)
            recip = sbuf.tile([128, 6], f32, tag="recip")
            nc.vector.reciprocal(recip, sums)
            Ph = Phb[b % 2]
            for h in range(6):
                nc.vector.tensor_scalar_mul(Ph[:, h, 0:SLAB], P[:, h, :],
                                            recip[:, h:h + 1])
            tiles["Ph", b] = Ph

        def emit_trans(b):
            Ph = tiles["Ph", b]
            PTa = sbuf.tile([128, 6, 128], f16, tag="PTa")
            Tb = sbuf.tile([16, 6, 128], f16, tag="Tb")
            tiles["PTa", b] = PTa
            tiles["Tb", b] = Tb
            for h in range(6):
                nc.tensor.matmul(
                    ps16[:, 1024 * h:1024 * h + 128],
                    lhsT=Ph[:, h, 0:128], rhs=ident,
                    is_transpose=True, start=True, stop=True)
            nc.vector.tensor_copy(PTa[:, :, :],
                                  apx(ps16[:, 0:128], [[1024, 6], [1, 128]]))
            for wv in range(3):
                for t in range(2):
                    nc.tensor.matmul(
                        ps16[0:W2, 1024 * (6 + t):1024 * (6 + t) + 128],
                        lhsT=Ph[:, 2 * wv + t, 128:128 + W2], rhs=ident,
                        is_transpose=True, start=True, stop=True)
                nc.vector.tensor_copy(
                    Tb[:, 2 * wv:2 * wv + 2, :],
                    apx(ps16[0:16, 0:128], [[1024, 2], [1, 128]],
                        extra_off=6 * 1024))

        def emit_pv_proj(b):
            PTa = tiles.pop(("PTa", b))
            Tb = tiles.pop(("Tb", b))
            tiles.pop(("E", b), None)
            tiles.pop(("Ph", b), None)
            aT = sbuf.tile([128, 3, 128], f16, tag="aT")
            for j in range(3):
                coff = 512 * (6 + (j & 1))
                h0, h1 = 2 * j, 2 * j + 1
                mm0 = nc.tensor.matmul(
                    PS[0:64, coff:coff + 128],
                    lhsT=V[:, b, 64 * h0:64 * h0 + 64],
                    rhs=PTa[:, h0, :], start=True, stop=False)
                mm1 = nc.tensor.matmul(
                    PS[0:64, coff:coff + 128],
                    lhsT=V[0:W2, b + 1, 64 * h0:64 * h0 + 64],
                    rhs=Tb[0:W2, h0, :], start=False, stop=True)
                mm2 = nc.tensor.matmul(
                    PS[64:128, coff:coff + 128],
                    lhsT=V[:, b, 64 * h1:64 * h1 + 64],
                    rhs=PTa[:, h1, :], start=True, stop=False)
                mm3 = nc.tensor.matmul(
                    PS[64:128, coff:coff + 128],
                    lhsT=V[0:W2, b + 1, 64 * h1:64 * h1 + 64],
                    rhs=Tb[0:W2, h1, :], start=False, stop=True)
                add_dep_helper(mm1.ins, mm0.ins, sync=False, reason="grp order")
                add_dep_helper(mm2.ins, mm1.ins, sync=False, reason="grp order")
                add_dep_helper(mm3.ins, mm2.ins, sync=False, reason="grp order")
                if j == 1:
                    nc.vector.tensor_copy(aT[:, j, :], PS[:, coff:coff + 128])
                else:
                    nc.scalar.copy(aT[:, j, :], PS[:, coff:coff + 128])
            yoff = 512 * 7
            for jj in range(3):
                nc.tensor.matmul(
                    PS[:, yoff:yoff + C],
                    lhsT=aT[:, jj, :], rhs=pw[:, jj, :],
                    start=(jj == 0), stop=(jj == 2 and not with_bias))
            if with_bias:
                nc.tensor.matmul(PS[:, yoff:yoff + C],
                                 lhsT=ones1[0:1, :], rhs=pb[0:1, :],
                                 start=False, stop=True)
            Yf = sbuf.tile([128, C], f16, tag="Yf")
            nc.vector.tensor_copy(Yf, PS[:, yoff:yoff + C])
            nc.sync.dma_start(out=out_d[128 * b:128 * (b + 1), :], in_=Yf)

        # 2-stage software pipeline: block b's scores/softmax run while
        # block b-1's transposes/PV/projection drain on banks 6,7
        def emit_v(r):
            rlen = min(128, RT - 128 * r)
            voff = 512 * (6 + (r & 1))
            for cc in range(3):
                nc.tensor.matmul(
                    PS[:rlen, voff:voff + C],
                    lhsT=xT[:, cc, 128 * r:128 * r + rlen],
                    rhs=vw[:, cc, :],
                    start=(cc == 0), stop=(cc == 2))
            if r % 2 == 0:
                nc.vector.tensor_copy(V[:rlen, r, :], PS[:rlen, voff:voff + C])
            else:
                nc.scalar.copy(V[:rlen, r, :], PS[:rlen, voff:voff + C])

        for b in range(NBLK):
            if b >= 1:
                emit_trans(b - 1)
            emit_scores(b)
            emit_exp(b)
            if b == 0:
                for r in range(NVCH):
                    emit_v(r)
            if b >= 1:
                emit_pv_proj(b - 1)
            emit_chain(b)
        emit_trans(NBLK - 1)
        emit_pv_proj(NBLK - 1)

    nc.finalize()
    return nc


def make_in_maps(x, qkv_w, proj_w, proj_b, w, with_bias):
    W2 = 2 * w
    RT = R + W2
    SLAB = 128 + W2

    qk_w = qkv_w[:2 * C].copy()
    qk_w[:C] *= SCALE                       # fold softmax scale into Q weights
    qk_wT = np.ascontiguousarray(qk_w.T).astype(np.float16)
    v_wT = np.ascontiguousarray(qkv_w[2 * C:].T).astype(np.float16)
    proj_wT = np.ascontiguousarray(proj_w.T).astype(np.float16)
    pb16 = proj_b.reshape(1, C).astype(np.float16)
    ident = np.eye(128, dtype=np.float16)

    p = np.arange(128)[:, None]
    t = np.arange(SLAB)[None, :]
    band = ((t >= p) & (t <= p + W2)).astype(np.float16)
    band = np.tile(band, (1, 6))

    in_maps = []
    for c in range(NCORES):
        b, s = divmod(c, SEQ_SHARDS)
        g0 = s * R
        xpad = np.zeros((RT, C), dtype=np.float32)
        lo = max(0, g0 - w)
        hi = min(N, g0 + R + w)
        xpad[lo - (g0 - w): hi - (g0 - w)] = x[b, lo:hi]
        xT = np.ascontiguousarray(xpad.T).astype(np.float16)
        kmask = np.zeros((1, RT), dtype=np.float16)
        if s == 0:
            kmask[0, :w] = NEG
        if s == SEQ_SHARDS - 1:
            kmask[0, R + w:] = NEG
        m = {"xT": xT, "qk_wT": qk_wT, "v_wT": v_wT, "proj_wT": proj_wT,
             "band": band, "kmask": kmask, "ident": ident}
        if with_bias:
            m["proj_b16"] = pb16
        in_maps.append(m)
    return in_maps


def kernel(x, qkv_w, proj_w, proj_b, epoch):
    x = np.asarray(x, dtype=np.float32)
    qkv_w = np.asarray(qkv_w, dtype=np.float32)
    proj_w = np.asarray(proj_w, dtype=np.float32)
    proj_b = np.asarray(proj_b, dtype=np.float32)
    w = _window_for_epoch(int(np.asarray(epoch)))
    if w is None or 128 + 2 * w > 512:
        return _numpy_fallback(x, qkv_w, proj_w, proj_b, w)

    from concourse.bass_utils import run_bass_kernel_spmd

    with_bias = bool(np.any(proj_b != 0.0))
    key = (w, with_bias)
    if key not in _PROGRAM_CACHE:
        _PROGRAM_CACHE[key] = build_program(w, with_bias)
    nc = _PROGRAM_CACHE[key]

    in_maps = make_in_maps(x, qkv_w, proj_w, proj_b, w, with_bias)
    res = run_bass_kernel_spmd(nc, in_maps, core_ids=list(range(NCORES)))

    out = np.empty((B, N, C), dtype=np.float32)
    for c in range(NCORES):
        b, s = divmod(c, SEQ_SHARDS)
        out[b, s * R:(s + 1) * R] = res.results[c]["out"].astype(np.float32)
    return out


# revision 40
# speedup vs baseline: 1.1979x; 1.0068x over previous
"""Banded (sliding-window) attention kernel for Trainium2, 8 NeuronCores.

Problem: nn_AttentionLAI (B=2, N=4096, C=384, H=6, head_dim=64), epoch=0
=> band window w=8 (each query attends keys with |i-j| <= 8).

Sharding: sequence sharding. core c = b*4 + s handles batch b, rows
[s*1024, (s+1)*1024) with a w-row halo on each side.  Zero collectives;
host gathers the 8 per-core outputs.

PSUM discipline (empirically validated on HW): at most ONE matmul output
column-range per 2KB bank at a time; writes at different partition
offsets to that same column range are fine; sequential reuse of a bank
with a different range is fine once prior readers complete.  The whole
PSUM is managed as one [128, 4096] f32 tile with manual bank regions.

Per-core dataflow (fp16 operands -> f32 PSUM accumulate):
  xT   [384, RT]   x-shard transposed (feature-major), RT = 1024 + 2w
  qkT  [768, RT]   = qk_w^T-chunks @ xT     (feature-major Q^T / K^T)
  V    [RT, 384]   = xT-chunks^T @ v_w^T    (row-major V)
  per 128-query block b (8 blocks):
    S_h  = Q^T_h' K^T_hslab -> PSUM bank h            (+ edge kmask acc)
    E    = exp(S)  (one ACT op, strided across banks)
    P    = E * band_mask;  sums = rowsum(P);  Phat = P / sums  (DVE)
    PT   = transpose(Phat) -> banks 0-5 main, 6-7 + 0-3 tails; bounce
    O^T  = V^T-chunks: 12 matmuls -> banks 4-6 (2 heads/bank partition-
           packed); bounce -> aT (feature-major, ready for projection)
    Y    = aT^T @ proj_w^T (+bias) -> bank 7; bounce; DMA out rows.
"""

import numpy as np

B, N, C, H = 2, 4096, 384, 6
HD = C // H            # 64
SCALE = HD ** -0.5
MASK_EPOCHS = [10, 20, 30, 40]
NCORES = 8
SEQ_SHARDS = 4         # per batch
R = N // SEQ_SHARDS    # 1024 rows per core
NBLK = R // 128        # 8 query blocks per core
NEG = -60000.0         # additive mask value (fits fp16, exp() -> 0)


def _window_for_epoch(epoch):
    if epoch >= MASK_EPOCHS[-1]:
        return None
    if epoch < MASK_EPOCHS[-4]:
        return 8
    elif epoch < MASK_EPOCHS[-3]:
        return 12
    elif epoch < MASK_EPOCHS[-2]:
        return 18
    else:
        return 20


def _numpy_fallback(x, qkv_w, proj_w, proj_b, w):
    """Reference-equivalent host computation (used only for epoch>=40)."""
    b, n, c = x.shape
    qkv = (x @ qkv_w.T).reshape(b, n, 3, H, HD).transpose(2, 0, 3, 1, 4)
    q, k, v = qkv[0], qkv[1], qkv[2]
    attn = np.einsum("bhnd,bhmd->bhnm", q, k) * SCALE
    if w is not None:
        idx = np.arange(n)
        band = np.abs(idx[:, None] - idx[None, :]) <= w
        attn = np.where(band[None, None], attn, -1e9)
    attn = attn - attn.max(axis=-1, keepdims=True)
    attn = np.exp(attn)
    attn = attn / attn.sum(axis=-1, keepdims=True)
    out = np.einsum("bhnm,bhmd->bhnd", attn, v)
    out = out.transpose(0, 2, 1, 3).reshape(b, n, c)
    return (out @ proj_w.T + proj_b).astype(np.float32)


_PROGRAM_CACHE = {}


def build_program(w, with_bias):
    """Build the SPMD Bass program for band window w. Returns nc."""
    import concourse.bass as bass
    import concourse.mybir as mybir
    import concourse.tile as tile
    from concourse import bacc
    from concourse.tile import add_dep_helper
    from contextlib import ExitStack

    f16 = mybir.dt.float16
    f32 = mybir.dt.float32
    AF = mybir.ActivationFunctionType
    AX = mybir.AxisListType

    W2 = 2 * w
    SLAB = 128 + W2            # keys per 128-query block
    RT = R + W2                # haloed rows per core
    assert SLAB <= 512
    NCH = [(0, 512), (512, 512), (1024, RT - 1024)]  # qkT col chunks
    NVCH = RT // 128 + (1 if RT % 128 else 0)        # V row chunks (9)

    nc = bacc.Bacc()

    xT_d = nc.declare_dram_parameter("xT", [C, RT], f16, isOutput=False)
    qkw_d = nc.declare_dram_parameter("qk_wT", [C, 2 * C], f16, isOutput=False)
    vw_d = nc.declare_dram_parameter("v_wT", [C, C], f16, isOutput=False)
    pw_d = nc.declare_dram_parameter("proj_wT", [C, C], f16, isOutput=False)
    band_d = nc.declare_dram_parameter("band", [128, 6 * SLAB], f16, isOutput=False)
    kmask_d = nc.declare_dram_parameter("kmask", [1, RT], f16, isOutput=False)
    ident_d = nc.declare_dram_parameter("ident", [128, 128], f16, isOutput=False)
    if with_bias:
        pb_d = nc.declare_dram_parameter("proj_b16", [1, C], f16, isOutput=False)
    out_d = nc.declare_dram_parameter("out", [R, C], f16, isOutput=True)

    def apx(sl, dims, extra_off=0):
        """AP with custom free dims on top of a slice's partition dim."""
        return bass.AP(tensor=sl.tensor, offset=sl.offset + extra_off,
                       ap=[sl.ap[0]] + dims)

    with ExitStack() as ctx:
        tc = ctx.enter_context(tile.TileContext(nc))
        consts = ctx.enter_context(tc.tile_pool(name="consts", bufs=1))

        xT = consts.tile([128, 3, RT], f16)
        qkw = consts.tile([128, 3, 2 * C], f16)
        vw = consts.tile([128, 3, C], f16)
        pw = consts.tile([128, 3, C], f16)
        band = consts.tile([128, 6, SLAB], f16)
        kmask = consts.tile([1, RT], f16)
        ident = consts.tile([128, 128], f16)
        ones1 = consts.tile([1, 128], f16)
        nc.vector.memset(ones1, 1.0)
        if with_bias:
            pb = consts.tile([1, C], f16)
            nc.sync.dma_start(out=pb, in_=pb_d[:, :])
        # spread input DMAs over four queues; phase-1's first deps come first
        half = RT // 2
        for j in range(3):
            nc.sync.dma_start(out=xT[:, j, 0:half],
                              in_=xT_d[128 * j:128 * (j + 1), 0:half])
            nc.scalar.dma_start(out=xT[:, j, half:RT],
                                in_=xT_d[128 * j:128 * (j + 1), half:RT])
            nc.gpsimd.dma_start(out=qkw[:, j, :], in_=qkw_d[128 * j:128 * (j + 1), :])
        for j in range(3):
            nc.gpsimd.dma_start(out=vw[:, j, :], in_=vw_d[128 * j:128 * (j + 1), :])
            nc.scalar.dma_start(out=pw[:, j, :], in_=pw_d[128 * j:128 * (j + 1), :])
        nc.scalar.dma_start(out=band[:, :, :], in_=band_d[:, :])
        nc.gpsimd.dma_start(out=kmask, in_=kmask_d[:, :])
        nc.gpsimd.dma_start(out=ident, in_=ident_d[:, :])

        qkT = consts.tile([128, 6, RT], f16)
        # per-head Q with the other head's 64 rows zeroed: lets the score
        # matmuls run at contract-128 / base-0 (walrus splits contract-64
        # matmuls into two HW instructions)
        qkQ = consts.tile([128, 6, RT], f16)
        for par in range(2):
            zsl = qkQ[64 * (1 - par):64 * (2 - par), par, :]
            nc.vector.memset(
                bass.AP(tensor=zsl.tensor, offset=zsl.offset,
                        ap=[zsl.ap[0], [2 * RT, 3], [1, RT]]), 0.0)
        V = consts.tile([128, NVCH, C], f16)
        # ping/pong normalized-score buffers; slot padded to 256 so the
        # 16-wide tail can be DMA-transposed as a full 128-col window
        Phb = []
        for i in range(2):
            Phx = consts.tile([128, 6, 256], f16, tag="Phb%d" % i)
            nc.gpsimd.memset(Phx[:, :, SLAB:256], 0.0)
            Phb.append(Phx)

        # ---- phase 1: qkT = qk_w^T @ x^T  (feature-major) ----
        eng = 0
        with ExitStack() as ph1:
            qkps = ph1.enter_context(tc.tile_pool(name="qkps", bufs=6, space="PSUM"))
            for (n0, ns) in NCH:
                for j in range(6):
                    ps = qkps.tile([128, 512], f32, tag="qkps")
                    for cc in range(3):
                        nc.tensor.matmul(
                            ps[:, :ns],
                            lhsT=qkw[:, cc, 128 * j:128 * (j + 1)],
                            rhs=xT[:, cc, n0:n0 + ns],
                            start=(cc == 0), stop=(cc == 2))
                    if j < 3:
                        nc.vector.tensor_copy(
                            qkQ[0:64, 2 * j, n0:n0 + ns], ps[0:64, :ns])
                        nc.scalar.copy(
                            qkQ[64:128, 2 * j + 1, n0:n0 + ns], ps[64:128, :ns])
                    elif eng % 2 == 0:
                        nc.vector.tensor_copy(qkT[:, j, n0:n0 + ns], ps[:, :ns])
                    else:
                        nc.scalar.copy(qkT[:, j, n0:n0 + ns], ps[:, :ns])
                    eng += 1


        # ---- phase 3: per-block attention + projection ----
        pspool = ctx.enter_context(tc.tile_pool(name="ps8", bufs=1, space="PSUM"))
        PS = pspool.tile([128, 4096], f32)          # the whole PSUM
        ps16 = PS[:, :].bitcast(f16)                # [128, 8192] fp16 view
        sbuf = ctx.enter_context(tc.tile_pool(name="work", bufs=2))

        tiles = {}

        def emit_scores(b, half):
            q0 = w + 128 * b
            k0 = 128 * b
            edge = b in (0, NBLK - 1)
            for h in range(3 * half, 3 * half + 3):
                nc.tensor.matmul(
                    PS[:, 512 * h:512 * h + SLAB],
                    lhsT=qkQ[:, h, q0:q0 + 128],
                    rhs=qkT[:, 3 + h // 2, k0:k0 + SLAB],
                    start=True, stop=not edge)
                if edge:
                    nc.tensor.matmul(
                        PS[:, 512 * h:512 * h + SLAB],
                        lhsT=ones1[0:1, :],
                        rhs=kmask[0:1, k0:k0 + SLAB],
                        start=False, stop=True)

        def emit_exp(b, half):
            if half == 0:
                E = sbuf.tile([128, 6, SLAB], f16, tag="E")
                tiles["E", b] = E
            E = tiles["E", b]
            nc.scalar.activation(
                out=E[:, 3 * half:3 * half + 3, :],
                in_=apx(PS[:, :], [[512, 3], [1, SLAB]],
                        extra_off=1536 * half),
                func=AF.Exp)

        def emit_chain(b, half):
            E = tiles["E", b]
            if half == 0:
                P = sbuf.tile([128, 6, SLAB], f16, tag="P")
                sums = sbuf.tile([128, 6], f32, tag="sums")
                recip = sbuf.tile([128, 6], f32, tag="recip")
                tiles["P", b] = P
                tiles["sums", b] = sums
                tiles["recip", b] = recip
                tiles["Ph", b] = Phb[b % 2]
            P = tiles["P", b]
            sums = tiles["sums", b]
            recip = tiles["recip", b]
            Ph = tiles["Ph", b]
            hs = slice(3 * half, 3 * half + 3)
            nc.vector.tensor_mul(P[:, hs, :], E[:, hs, :], band[:, hs, :])
            nc.vector.reduce_sum(out=sums[:, hs], in_=P[:, hs, :], axis=AX.X)
            nc.vector.reciprocal(recip[:, hs], sums[:, hs])
            for h in range(3 * half, 3 * half + 3):
                nc.vector.tensor_scalar_mul(Ph[:, h, 0:SLAB], P[:, h, :],
                                            recip[:, h:h + 1])

        def emit_trans(b):
            Ph = tiles["Ph", b]
            PTa = sbuf.tile([128, 6, 128], f16, tag="PTa")
            Tb = sbuf.tile([16, 6, 128], f16, tag="Tb")
            tiles["PTa", b] = PTa
            tiles["Tb", b] = Tb
            for h in range(6):
                nc.tensor.matmul(
                    ps16[:, 1024 * h:1024 * h + 128],
                    lhsT=Ph[:, h, 0:128], rhs=ident,
                    is_transpose=True, start=True, stop=True)
            nc.vector.tensor_copy(PTa[:, :, :],
                                  apx(ps16[:, 0:128], [[1024, 6], [1, 128]]))
            for wv in range(3):
                for t in range(2):
                    nc.tensor.matmul(
                        ps16[0:W2, 1024 * (6 + t):1024 * (6 + t) + 128],
                        lhsT=Ph[:, 2 * wv + t, 128:128 + W2], rhs=ident,
                        is_transpose=True, start=True, stop=True)
                nc.vector.tensor_copy(
                    Tb[:, 2 * wv:2 * wv + 2, :],
                    apx(ps16[0:16, 0:128], [[1024, 2], [1, 128]],
                        extra_off=6 * 1024))

        def emit_pv_proj(b):
            PTa = tiles.pop(("PTa", b))
            Tb = tiles.pop(("Tb", b))
            tiles.pop(("E", b), None)
            tiles.pop(("Ph", b), None)
            aT = sbuf.tile([128, 3, 128], f16, tag="aT")
            for j in range(3):
                coff = 512 * (6 + (j & 1))
                h0, h1 = 2 * j, 2 * j + 1
                mm0 = nc.tensor.matmul(
                    PS[0:64, coff:coff + 128],
                    lhsT=V[:, b, 64 * h0:64 * h0 + 64],
                    rhs=PTa[:, h0, :], start=True, stop=False)
                mm1 = nc.tensor.matmul(
                    PS[0:64, coff:coff + 128],
                    lhsT=V[0:W2, b + 1, 64 * h0:64 * h0 + 64],
                    rhs=Tb[0:W2, h0, :], start=False, stop=True)
                mm2 = nc.tensor.matmul(
                    PS[64:128, coff:coff + 128],
                    lhsT=V[:, b, 64 * h1:64 * h1 + 64],
                    rhs=PTa[:, h1, :], start=True, stop=False)
                mm3 = nc.tensor.matmul(
                    PS[64:128, coff:coff + 128],
                    lhsT=V[0:W2, b + 1, 64 * h1:64 * h1 + 64],
                    rhs=Tb[0:W2, h1, :], start=False, stop=True)
                add_dep_helper(mm1.ins, mm0.ins, sync=False, reason="grp order")
                add_dep_helper(mm2.ins, mm1.ins, sync=False, reason="grp order")
                add_dep_helper(mm3.ins, mm2.ins, sync=False, reason="grp order")
                if j == 1:
                    nc.vector.tensor_copy(aT[:, j, :], PS[:, coff:coff + 128])
                else:
                    nc.scalar.copy(aT[:, j, :], PS[:, coff:coff + 128])
            yoff = 512 * 7
            for jj in range(3):
                nc.tensor.matmul(
                    PS[:, yoff:yoff + C],
                    lhsT=aT[:, jj, :], rhs=pw[:, jj, :],
                    start=(jj == 0), stop=(jj == 2 and not with_bias))
            if with_bias:
                nc.tensor.matmul(PS[:, yoff:yoff + C],
                                 lhsT=ones1[0:1, :], rhs=pb[0:1, :],
                                 start=False, stop=True)
            Yf = sbuf.tile([128, C], f16, tag="Yf")
            nc.vector.tensor_copy(Yf, PS[:, yoff:yoff + C])
            nc.sync.dma_start(out=out_d[128 * b:128 * (b + 1), :], in_=Yf)

        # 2-stage software pipeline: block b's scores/softmax run while
        # block b-1's transposes/PV/projection drain on banks 6,7
        def emit_v(r):
            rlen = min(128, RT - 128 * r)
            voff = 512 * (6 + (r & 1))
            for cc in range(3):
                nc.tensor.matmul(
                    PS[:rlen, voff:voff + C],
                    lhsT=xT[:, cc, 128 * r:128 * r + rlen],
                    rhs=vw[:, cc, :],
                    start=(cc == 0), stop=(cc == 2))
            if r % 2 == 0:
                nc.vector.tensor_copy(V[:rlen, r, :], PS[:rlen, voff:voff + C])
            else:
                nc.scalar.copy(V[:rlen, r, :], PS[:rlen, voff:voff + C])

        for b in range(NBLK):
            if b >= 1:
                emit_trans(b - 1)
            emit_scores(b, 0)
            emit_exp(b, 0)
            emit_scores(b, 1)
            emit_chain(b, 0)
            emit_exp(b, 1)
            if b == 0:
                for r in range(NVCH):
                    emit_v(r)
            if b >= 1:
                emit_pv_proj(b - 1)
            emit_chain(b, 1)
        emit_trans(NBLK - 1)
        emit_pv_proj(NBLK - 1)

    nc.finalize()
    return nc


def make_in_maps(x, qkv_w, proj_w, proj_b, w, with_bias):
    W2 = 2 * w
    RT = R + W2
    SLAB = 128 + W2

    qk_w = qkv_w[:2 * C].copy()
    qk_w[:C] *= SCALE                       # fold softmax scale into Q weights
    qk_wT = np.ascontiguousarray(qk_w.T).astype(np.float16)
    v_wT = np.ascontiguousarray(qkv_w[2 * C:].T).astype(np.float16)
    proj_wT = np.ascontiguousarray(proj_w.T).astype(np.float16)
    pb16 = proj_b.reshape(1, C).astype(np.float16)
    ident = np.eye(128, dtype=np.float16)

    p = np.arange(128)[:, None]
    t = np.arange(SLAB)[None, :]
    band = ((t >= p) & (t <= p + W2)).astype(np.float16)
    band = np.tile(band, (1, 6))

    in_maps = []
    for c in range(NCORES):
        b, s = divmod(c, SEQ_SHARDS)
        g0 = s * R
        xpad = np.zeros((RT, C), dtype=np.float32)
        lo = max(0, g0 - w)
        hi = min(N, g0 + R + w)
        xpad[lo - (g0 - w): hi - (g0 - w)] = x[b, lo:hi]
        xT = np.ascontiguousarray(xpad.T).astype(np.float16)
        kmask = np.zeros((1, RT), dtype=np.float16)
        if s == 0:
            kmask[0, :w] = NEG
        if s == SEQ_SHARDS - 1:
            kmask[0, R + w:] = NEG
        m = {"xT": xT, "qk_wT": qk_wT, "v_wT": v_wT, "proj_wT": proj_wT,
             "band": band, "kmask": kmask, "ident": ident}
        if with_bias:
            m["proj_b16"] = pb16
        in_maps.append(m)
    return in_maps


def kernel(x, qkv_w, proj_w, proj_b, epoch):
    x = np.asarray(x, dtype=np.float32)
    qkv_w = np.asarray(qkv_w, dtype=np.float32)
    proj_w = np.asarray(proj_w, dtype=np.float32)
    proj_b = np.asarray(proj_b, dtype=np.float32)
    w = _window_for_epoch(int(np.asarray(epoch)))
    if w is None or 128 + 2 * w > 512:
        return _numpy_fallback(x, qkv_w, proj_w, proj_b, w)

    from concourse.bass_utils import run_bass_kernel_spmd

    with_bias = bool(np.any(proj_b != 0.0))
    key = (w, with_bias)
    if key not in _PROGRAM_CACHE:
        _PROGRAM_CACHE[key] = build_program(w, with_bias)
    nc = _PROGRAM_CACHE[key]

    in_maps = make_in_maps(x, qkv_w, proj_w, proj_b, w, with_bias)
    res = run_bass_kernel_spmd(nc, in_maps, core_ids=list(range(NCORES)))

    out = np.empty((B, N, C), dtype=np.float32)
    for c in range(NCORES):
        b, s = divmod(c, SEQ_SHARDS)
        out[b, s * R:(s + 1) * R] = res.results[c]["out"].astype(np.float32)
    return out


# revision 41
# speedup vs baseline: 1.2508x; 1.0441x over previous
"""Banded (sliding-window) attention kernel for Trainium2, 8 NeuronCores.

Problem: nn_AttentionLAI (B=2, N=4096, C=384, H=6, head_dim=64), epoch=0
=> band window w=8 (each query attends keys with |i-j| <= 8).

Sharding: sequence sharding. core c = b*4 + s handles batch b, rows
[s*1024, (s+1)*1024) with a w-row halo on each side.  Zero collectives;
host gathers the 8 per-core outputs.

PSUM discipline (empirically validated on HW): at most ONE matmul output
column-range per 2KB bank at a time; writes at different partition
offsets to that same column range are fine; sequential reuse of a bank
with a different range is fine once prior readers complete.  The whole
PSUM is managed as one [128, 4096] f32 tile with manual bank regions.

Per-core dataflow (fp16 operands -> f32 PSUM accumulate):
  xT   [384, RT]   x-shard transposed (feature-major), RT = 1024 + 2w
  qkT  [768, RT]   = qk_w^T-chunks @ xT     (feature-major Q^T / K^T)
  V    [RT, 384]   = xT-chunks^T @ v_w^T    (row-major V)
  per 128-query block b (8 blocks):
    S_h  = Q^T_h' K^T_hslab -> PSUM bank h            (+ edge kmask acc)
    E    = exp(S)  (one ACT op, strided across banks)
    P    = E * band_mask;  sums = rowsum(P);  Phat = P / sums  (DVE)
    PT   = transpose(Phat) -> banks 0-5 main, 6-7 + 0-3 tails; bounce
    O^T  = V^T-chunks: 12 matmuls -> banks 4-6 (2 heads/bank partition-
           packed); bounce -> aT (feature-major, ready for projection)
    Y    = aT^T @ proj_w^T (+bias) -> bank 7; bounce; DMA out rows.
"""

import numpy as np

B, N, C, H = 2, 4096, 384, 6
HD = C // H            # 64
SCALE = HD ** -0.5
MASK_EPOCHS = [10, 20, 30, 40]
NCORES = 8
SEQ_SHARDS = 4         # per batch
R = N // SEQ_SHARDS    # 1024 rows per core
NBLK = R // 128        # 8 query blocks per core
NEG = -60000.0         # additive mask value (fits fp16, exp() -> 0)


def _window_for_epoch(epoch):
    if epoch >= MASK_EPOCHS[-1]:
        return None
    if epoch < MASK_EPOCHS[-4]:
        return 8
    elif epoch < MASK_EPOCHS[-3]:
        return 12
    elif epoch < MASK_EPOCHS[-2]:
        return 18
    else:
        return 20


def _numpy_fallback(x, qkv_w, proj_w, proj_b, w):
    """Reference-equivalent host computation (used only for epoch>=40)."""
    b, n, c = x.shape
    qkv = (x @ qkv_w.T).reshape(b, n, 3, H, HD).transpose(2, 0, 3, 1, 4)
    q, k, v = qkv[0], qkv[1], qkv[2]
    attn = np.einsum("bhnd,bhmd->bhnm", q, k) * SCALE
    if w is not None:
        idx = np.arange(n)
        band = np.abs(idx[:, None] - idx[None, :]) <= w
        attn = np.where(band[None, None], attn, -1e9)
    attn = attn - attn.max(axis=-1, keepdims=True)
    attn = np.exp(attn)
    attn = attn / attn.sum(axis=-1, keepdims=True)
    out = np.einsum("bhnm,bhmd->bhnd", attn, v)
    out = out.transpose(0, 2, 1, 3).reshape(b, n, c)
    return (out @ proj_w.T + proj_b).astype(np.float32)


_PROGRAM_CACHE = {}


def build_program(w, with_bias):
    """Build the SPMD Bass program for band window w. Returns nc."""
    import concourse.bass as bass
    import concourse.mybir as mybir
    import concourse.tile as tile
    from concourse import bacc
    from concourse.tile import add_dep_helper
    from contextlib import ExitStack

    f16 = mybir.dt.float16
    f32 = mybir.dt.float32
    AF = mybir.ActivationFunctionType
    AX = mybir.AxisListType

    W2 = 2 * w
    SLAB = 128 + W2            # keys per 128-query block
    RT = R + W2                # haloed rows per core
    assert SLAB <= 512
    NCH = [(0, 512), (512, 512), (1024, RT - 1024)]  # qkT col chunks
    NVCH = RT // 128 + (1 if RT % 128 else 0)        # V row chunks (9)

    nc = bacc.Bacc()

    xT_d = nc.declare_dram_parameter("xT", [C, RT], f16, isOutput=False)
    qkw_d = nc.declare_dram_parameter("qk_wT", [C, 2 * C], f16, isOutput=False)
    vw_d = nc.declare_dram_parameter("v_wT", [C, C], f16, isOutput=False)
    pw_d = nc.declare_dram_parameter("proj_wT", [C, C], f16, isOutput=False)
    band_d = nc.declare_dram_parameter("band", [3, 128, 6 * SLAB], f16, isOutput=False)
    kmask_d = nc.declare_dram_parameter("kmask", [1, RT], f16, isOutput=False)
    ident_d = nc.declare_dram_parameter("ident", [128, 128], f16, isOutput=False)
    if with_bias:
        pb_d = nc.declare_dram_parameter("proj_b16", [1, C], f16, isOutput=False)
    out_d = nc.declare_dram_parameter("out", [R, C], f16, isOutput=True)

    def apx(sl, dims, extra_off=0):
        """AP with custom free dims on top of a slice's partition dim."""
        return bass.AP(tensor=sl.tensor, offset=sl.offset + extra_off,
                       ap=[sl.ap[0]] + dims)

    with ExitStack() as ctx:
        tc = ctx.enter_context(tile.TileContext(nc))
        consts = ctx.enter_context(tc.tile_pool(name="consts", bufs=1))

        xT = consts.tile([128, 3, RT], f16)
        qkw = consts.tile([128, 3, 2 * C], f16)
        vw = consts.tile([128, 3, C], f16)
        pw = consts.tile([128, 3, C], f16)
        band = consts.tile([128, 3, 6, SLAB], f16)
        kmask = consts.tile([1, RT], f16)
        ident = consts.tile([128, 128], f16)
        ones1 = consts.tile([1, 128], f16)
        nc.vector.memset(ones1, 1.0)
        if with_bias:
            pb = consts.tile([1, C], f16)
            nc.sync.dma_start(out=pb, in_=pb_d[:, :])
        # spread input DMAs over four queues; phase-1's first deps come first
        half = RT // 2
        for j in range(3):
            nc.sync.dma_start(out=xT[:, j, 0:half],
                              in_=xT_d[128 * j:128 * (j + 1), 0:half])
            nc.scalar.dma_start(out=xT[:, j, half:RT],
                                in_=xT_d[128 * j:128 * (j + 1), half:RT])
            nc.gpsimd.dma_start(out=qkw[:, j, :], in_=qkw_d[128 * j:128 * (j + 1), :])
        for j in range(3):
            nc.gpsimd.dma_start(out=vw[:, j, :], in_=vw_d[128 * j:128 * (j + 1), :])
            nc.scalar.dma_start(out=pw[:, j, :], in_=pw_d[128 * j:128 * (j + 1), :])
        for v in range(3):
            nc.scalar.dma_start(out=band[:, v, :, :], in_=band_d[v, :, :])
        nc.gpsimd.dma_start(out=kmask, in_=kmask_d[:, :])
        nc.gpsimd.dma_start(out=ident, in_=ident_d[:, :])

        qkT = consts.tile([128, 6, RT], f16)
        # per-head Q with the other head's 64 rows zeroed: lets the score
        # matmuls run at contract-128 / base-0 (walrus splits contract-64
        # matmuls into two HW instructions)
        qkQ = consts.tile([128, 6, RT], f16)
        for par in range(2):
            zsl = qkQ[64 * (1 - par):64 * (2 - par), par, :]
            nc.vector.memset(
                bass.AP(tensor=zsl.tensor, offset=zsl.offset,
                        ap=[zsl.ap[0], [2 * RT, 3], [1, RT]]), 0.0)
        V = consts.tile([128, NVCH, C], f16)
        # ping/pong normalized-score buffers; slot padded to 256 so the
        # 16-wide tail can be DMA-transposed as a full 128-col window
        Phb = []
        for i in range(2):
            Phx = consts.tile([128, 6, 256], f16, tag="Phb%d" % i)
            nc.gpsimd.memset(Phx[:, :, SLAB:256], 0.0)
            Phb.append(Phx)

        # ---- phase 1: qkT = qk_w^T @ x^T  (feature-major) ----
        eng = 0
        with ExitStack() as ph1:
            qkps = ph1.enter_context(tc.tile_pool(name="qkps", bufs=6, space="PSUM"))
            for (n0, ns) in NCH:
                for j in range(6):
                    ps = qkps.tile([128, 512], f32, tag="qkps")
                    for cc in range(3):
                        nc.tensor.matmul(
                            ps[:, :ns],
                            lhsT=qkw[:, cc, 128 * j:128 * (j + 1)],
                            rhs=xT[:, cc, n0:n0 + ns],
                            start=(cc == 0), stop=(cc == 2))
                    if j < 3:
                        nc.vector.tensor_copy(
                            qkQ[0:64, 2 * j, n0:n0 + ns], ps[0:64, :ns])
                        nc.scalar.copy(
                            qkQ[64:128, 2 * j + 1, n0:n0 + ns], ps[64:128, :ns])
                    elif eng % 2 == 0:
                        nc.vector.tensor_copy(qkT[:, j, n0:n0 + ns], ps[:, :ns])
                    else:
                        nc.scalar.copy(qkT[:, j, n0:n0 + ns], ps[:, :ns])
                    eng += 1


        # ---- phase 3: per-block attention + projection ----
        pspool = ctx.enter_context(tc.tile_pool(name="ps8", bufs=1, space="PSUM"))
        PS = pspool.tile([128, 4096], f32)          # the whole PSUM
        ps16 = PS[:, :].bitcast(f16)                # [128, 8192] fp16 view
        sbuf = ctx.enter_context(tc.tile_pool(name="work", bufs=2))

        tiles = {}

        def emit_scores(b, half):
            q0 = w + 128 * b
            k0 = 128 * b
            edge = b in (0, NBLK - 1)
            for h in range(3 * half, 3 * half + 3):
                nc.tensor.matmul(
                    PS[:, 512 * h:512 * h + SLAB],
                    lhsT=qkQ[:, h, q0:q0 + 128],
                    rhs=qkT[:, 3 + h // 2, k0:k0 + SLAB],
                    start=True, stop=True)

        def emit_exp(b, half):
            if half == 0:
                E = sbuf.tile([128, 6, SLAB], f16, tag="E")
                tiles["E", b] = E
            E = tiles["E", b]
            nc.scalar.activation(
                out=E[:, 3 * half:3 * half + 3, :],
                in_=apx(PS[:, :], [[512, 3], [1, SLAB]],
                        extra_off=1536 * half),
                func=AF.Exp)

        def emit_chain(b, half):
            E = tiles["E", b]
            if half == 0:
                P = sbuf.tile([128, 6, SLAB], f16, tag="P")
                sums = sbuf.tile([128, 6], f32, tag="sums")
                recip = sbuf.tile([128, 6], f32, tag="recip")
                tiles["P", b] = P
                tiles["sums", b] = sums
                tiles["recip", b] = recip
                tiles["Ph", b] = Phb[b % 2]
            P = tiles["P", b]
            sums = tiles["sums", b]
            recip = tiles["recip", b]
            Ph = tiles["Ph", b]
            hs = slice(3 * half, 3 * half + 3)
            bv = 0 if b == 0 else (2 if b == NBLK - 1 else 1)
            nc.vector.tensor_mul(P[:, hs, :], E[:, hs, :], band[:, bv, hs, :])
            nc.vector.reduce_sum(out=sums[:, hs], in_=P[:, hs, :], axis=AX.X)
            nc.vector.reciprocal(recip[:, hs], sums[:, hs])
            for h in range(3 * half, 3 * half + 3):
                nc.vector.tensor_scalar_mul(Ph[:, h, 0:SLAB], P[:, h, :],
                                            recip[:, h:h + 1])

        def emit_trans(b):
            Ph = tiles["Ph", b]
            PTa = sbuf.tile([128, 6, 128], f16, tag="PTa")
            Tb = sbuf.tile([16, 6, 128], f16, tag="Tb")
            tiles["PTa", b] = PTa
            tiles["Tb", b] = Tb
            for h in range(6):
                nc.tensor.matmul(
                    ps16[:, 1024 * h:1024 * h + 128],
                    lhsT=Ph[:, h, 0:128], rhs=ident,
                    is_transpose=True, start=True, stop=True)
            nc.vector.tensor_copy(PTa[:, :, :],
                                  apx(ps16[:, 0:128], [[1024, 6], [1, 128]]))
            for wv in range(3):
                for t in range(2):
                    nc.tensor.matmul(
                        ps16[0:W2, 1024 * (6 + t):1024 * (6 + t) + 128],
                        lhsT=Ph[:, 2 * wv + t, 128:128 + W2], rhs=ident,
                        is_transpose=True, start=True, stop=True)
                nc.vector.tensor_copy(
                    Tb[:, 2 * wv:2 * wv + 2, :],
                    apx(ps16[0:16, 0:128], [[1024, 2], [1, 128]],
                        extra_off=6 * 1024))

        def emit_pv_proj(b):
            PTa = tiles.pop(("PTa", b))
            Tb = tiles.pop(("Tb", b))
            tiles.pop(("E", b), None)
            tiles.pop(("Ph", b), None)
            aT = sbuf.tile([128, 3, 128], f16, tag="aT")
            for j in range(3):
                coff = 512 * (6 + (j & 1))
                h0, h1 = 2 * j, 2 * j + 1
                mm0 = nc.tensor.matmul(
                    PS[0:64, coff:coff + 128],
                    lhsT=V[:, b, 64 * h0:64 * h0 + 64],
                    rhs=PTa[:, h0, :], start=True, stop=False)
                mm1 = nc.tensor.matmul(
                    PS[0:64, coff:coff + 128],
                    lhsT=V[0:W2, b + 1, 64 * h0:64 * h0 + 64],
                    rhs=Tb[0:W2, h0, :], start=False, stop=True)
                mm2 = nc.tensor.matmul(
                    PS[64:128, coff:coff + 128],
                    lhsT=V[:, b, 64 * h1:64 * h1 + 64],
                    rhs=PTa[:, h1, :], start=True, stop=False)
                mm3 = nc.tensor.matmul(
                    PS[64:128, coff:coff + 128],
                    lhsT=V[0:W2, b + 1, 64 * h1:64 * h1 + 64],
                    rhs=Tb[0:W2, h1, :], start=False, stop=True)
                add_dep_helper(mm1.ins, mm0.ins, sync=False, reason="grp order")
                add_dep_helper(mm2.ins, mm1.ins, sync=False, reason="grp order")
                add_dep_helper(mm3.ins, mm2.ins, sync=False, reason="grp order")
                if j == 1:
                    nc.vector.tensor_copy(aT[:, j, :], PS[:, coff:coff + 128])
                else:
                    nc.scalar.copy(aT[:, j, :], PS[:, coff:coff + 128])
            yoff = 512 * 7
            for jj in range(3):
                nc.tensor.matmul(
                    PS[:, yoff:yoff + C],
                    lhsT=aT[:, jj, :], rhs=pw[:, jj, :],
                    start=(jj == 0), stop=(jj == 2 and not with_bias))
            if with_bias:
                nc.tensor.matmul(PS[:, yoff:yoff + C],
                                 lhsT=ones1[0:1, :], rhs=pb[0:1, :],
                                 start=False, stop=True)
            Yf = sbuf.tile([128, C], f16, tag="Yf")
            nc.vector.tensor_copy(Yf, PS[:, yoff:yoff + C])
            nc.sync.dma_start(out=out_d[128 * b:128 * (b + 1), :], in_=Yf)

        # 2-stage software pipeline: block b's scores/softmax run while
        # block b-1's transposes/PV/projection drain on banks 6,7
        def emit_v(r):
            rlen = min(128, RT - 128 * r)
            voff = 512 * (6 + (r & 1))
            for cc in range(3):
                nc.tensor.matmul(
                    PS[:rlen, voff:voff + C],
                    lhsT=xT[:, cc, 128 * r:128 * r + rlen],
                    rhs=vw[:, cc, :],
                    start=(cc == 0), stop=(cc == 2))
            if r % 2 == 0:
                nc.vector.tensor_copy(V[:rlen, r, :], PS[:rlen, voff:voff + C])
            else:
                nc.scalar.copy(V[:rlen, r, :], PS[:rlen, voff:voff + C])

        for b in range(NBLK):
            if b >= 1:
                emit_trans(b - 1)
            emit_scores(b, 0)
            emit_exp(b, 0)
            emit_scores(b, 1)
            emit_chain(b, 0)
            emit_exp(b, 1)
            if b == 0:
                for r in range(NVCH):
                    emit_v(r)
            if b >= 1:
                emit_pv_proj(b - 1)
            emit_chain(b, 1)
        emit_trans(NBLK - 1)
        emit_pv_proj(NBLK - 1)

    nc.finalize()
    return nc


def make_in_maps(x, qkv_w, proj_w, proj_b, w, with_bias):
    W2 = 2 * w
    RT = R + W2
    SLAB = 128 + W2

    qk_w = qkv_w[:2 * C].copy()
    qk_w[:C] *= SCALE                       # fold softmax scale into Q weights
    qk_wT = np.ascontiguousarray(qk_w.T).astype(np.float16)
    v_wT = np.ascontiguousarray(qkv_w[2 * C:].T).astype(np.float16)
    proj_wT = np.ascontiguousarray(proj_w.T).astype(np.float16)
    pb16 = proj_b.reshape(1, C).astype(np.float16)
    ident = np.eye(128, dtype=np.float16)

    p = np.arange(128)[:, None]
    t = np.arange(SLAB)[None, :]
    band = ((t >= p) & (t <= p + W2)).astype(np.float16)

    in_maps = []
    for c in range(NCORES):
        b, s = divmod(c, SEQ_SHARDS)
        g0 = s * R
        xpad = np.zeros((RT, C), dtype=np.float32)
        lo = max(0, g0 - w)
        hi = min(N, g0 + R + w)
        xpad[lo - (g0 - w): hi - (g0 - w)] = x[b, lo:hi]
        xT = np.ascontiguousarray(xpad.T).astype(np.float16)
        kmask = np.zeros((1, RT), dtype=np.float16)
        bandF = band.copy()
        bandL = band.copy()
        if s == 0:
            bandF[:, :w] = 0.0          # keys with global j < 0
        if s == SEQ_SHARDS - 1:
            bandL[:, SLAB - w:] = 0.0   # keys with global j >= N
        band3 = np.stack([np.tile(bv, (1, 6)) for bv in (bandF, band, bandL)])
        m = {"xT": xT, "qk_wT": qk_wT, "v_wT": v_wT, "proj_wT": proj_wT,
             "band": band3, "kmask": kmask, "ident": ident}
        if with_bias:
            m["proj_b16"] = pb16
        in_maps.append(m)
    return in_maps


def kernel(x, qkv_w, proj_w, proj_b, epoch):
    x = np.asarray(x, dtype=np.float32)
    qkv_w = np.asarray(qkv_w, dtype=np.float32)
    proj_w = np.asarray(proj_w, dtype=np.float32)
    proj_b = np.asarray(proj_b, dtype=np.float32)
    w = _window_for_epoch(int(np.asarray(epoch)))
    if w is None or 128 + 2 * w > 512:
        return _numpy_fallback(x, qkv_w, proj_w, proj_b, w)

    from concourse.bass_utils import run_bass_kernel_spmd

    with_bias = bool(np.any(proj_b != 0.0))
    key = (w, with_bias)
    if key not in _PROGRAM_CACHE:
        _PROGRAM_CACHE[key] = build_program(w, with_bias)
    nc = _PROGRAM_CACHE[key]

    in_maps = make_in_maps(x, qkv_w, proj_w, proj_b, w, with_bias)
    res = run_bass_kernel_spmd(nc, in_maps, core_ids=list(range(NCORES)))

    out = np.empty((B, N, C), dtype=np.float32)
    for c in range(NCORES):
        b, s = divmod(c, SEQ_SHARDS)
        out[b, s * R:(s + 1) * R] = res.results[c]["out"].astype(np.float32)
    return out


# revision 42
# speedup vs baseline: 1.2542x; 1.0027x over previous
"""Banded (sliding-window) attention kernel for Trainium2, 8 NeuronCores.

Problem: nn_AttentionLAI (B=2, N=4096, C=384, H=6, head_dim=64), epoch=0
=> band window w=8 (each query attends keys with |i-j| <= 8).

Sharding: sequence sharding. core c = b*4 + s handles batch b, rows
[s*1024, (s+1)*1024) with a w-row halo on each side.  Zero collectives;
host gathers the 8 per-core outputs.

PSUM discipline (empirically validated on HW): at most ONE matmul output
column-range per 2KB bank at a time; writes at different partition
offsets to that same column range are fine; sequential reuse of a bank
with a different range is fine once prior readers complete.  The whole
PSUM is managed as one [128, 4096] f32 tile with manual bank regions.

Per-core dataflow (fp16 operands -> f32 PSUM accumulate):
  xT   [384, RT]   x-shard transposed (feature-major), RT = 1024 + 2w
  qkT  [768, RT]   = qk_w^T-chunks @ xT     (feature-major Q^T / K^T)
  V    [RT, 384]   = xT-chunks^T @ v_w^T    (row-major V)
  per 128-query block b (8 blocks):
    S_h  = Q^T_h' K^T_hslab -> PSUM bank h            (+ edge kmask acc)
    E    = exp(S)  (one ACT op, strided across banks)
    P    = E * band_mask;  sums = rowsum(P);  Phat = P / sums  (DVE)
    PT   = transpose(Phat) -> banks 0-5 main, 6-7 + 0-3 tails; bounce
    O^T  = V^T-chunks: 12 matmuls -> banks 4-6 (2 heads/bank partition-
           packed); bounce -> aT (feature-major, ready for projection)
    Y    = aT^T @ proj_w^T (+bias) -> bank 7; bounce; DMA out rows.
"""

import numpy as np

B, N, C, H = 2, 4096, 384, 6
HD = C // H            # 64
SCALE = HD ** -0.5
MASK_EPOCHS = [10, 20, 30, 40]
NCORES = 8
SEQ_SHARDS = 4         # per batch
R = N // SEQ_SHARDS    # 1024 rows per core
NBLK = R // 128        # 8 query blocks per core
NEG = -60000.0         # additive mask value (fits fp16, exp() -> 0)


def _window_for_epoch(epoch):
    if epoch >= MASK_EPOCHS[-1]:
        return None
    if epoch < MASK_EPOCHS[-4]:
        return 8
    elif epoch < MASK_EPOCHS[-3]:
        return 12
    elif epoch < MASK_EPOCHS[-2]:
        return 18
    else:
        return 20


def _numpy_fallback(x, qkv_w, proj_w, proj_b, w):
    """Reference-equivalent host computation (used only for epoch>=40)."""
    b, n, c = x.shape
    qkv = (x @ qkv_w.T).reshape(b, n, 3, H, HD).transpose(2, 0, 3, 1, 4)
    q, k, v = qkv[0], qkv[1], qkv[2]
    attn = np.einsum("bhnd,bhmd->bhnm", q, k) * SCALE
    if w is not None:
        idx = np.arange(n)
        band = np.abs(idx[:, None] - idx[None, :]) <= w
        attn = np.where(band[None, None], attn, -1e9)
    attn = attn - attn.max(axis=-1, keepdims=True)
    attn = np.exp(attn)
    attn = attn / attn.sum(axis=-1, keepdims=True)
    out = np.einsum("bhnm,bhmd->bhnd", attn, v)
    out = out.transpose(0, 2, 1, 3).reshape(b, n, c)
    return (out @ proj_w.T + proj_b).astype(np.float32)


_PROGRAM_CACHE = {}


def build_program(w, with_bias):
    """Build the SPMD Bass program for band window w. Returns nc."""
    import concourse.bass as bass
    import concourse.mybir as mybir
    import concourse.tile as tile
    from concourse import bacc
    from concourse.tile import add_dep_helper
    from contextlib import ExitStack

    f16 = mybir.dt.float16
    f32 = mybir.dt.float32
    AF = mybir.ActivationFunctionType
    AX = mybir.AxisListType

    W2 = 2 * w
    SLAB = 128 + W2            # keys per 128-query block
    RT = R + W2                # haloed rows per core
    assert SLAB <= 512
    NCH = [(0, 512), (512, 512), (1024, RT - 1024)]  # qkT col chunks
    NVCH = RT // 128 + (1 if RT % 128 else 0)        # V row chunks (9)

    nc = bacc.Bacc()

    xT_d = nc.declare_dram_parameter("xT", [C, RT], f16, isOutput=False)
    qkw_d = nc.declare_dram_parameter("qk_wT", [C, 2 * C], f16, isOutput=False)
    vw_d = nc.declare_dram_parameter("v_wT", [C, C], f16, isOutput=False)
    pw_d = nc.declare_dram_parameter("proj_wT", [C, C], f16, isOutput=False)
    band_d = nc.declare_dram_parameter("band", [3, 128, 6 * SLAB], f16, isOutput=False)
    kmask_d = nc.declare_dram_parameter("kmask", [1, RT], f16, isOutput=False)
    ident_d = nc.declare_dram_parameter("ident", [128, 128], f16, isOutput=False)
    if with_bias:
        pb_d = nc.declare_dram_parameter("proj_b16", [1, C], f16, isOutput=False)
    out_d = nc.declare_dram_parameter("out", [R, C], f16, isOutput=True)

    def apx(sl, dims, extra_off=0):
        """AP with custom free dims on top of a slice's partition dim."""
        return bass.AP(tensor=sl.tensor, offset=sl.offset + extra_off,
                       ap=[sl.ap[0]] + dims)

    with ExitStack() as ctx:
        tc = ctx.enter_context(tile.TileContext(nc))
        consts = ctx.enter_context(tc.tile_pool(name="consts", bufs=1))

        xT = consts.tile([128, 3, RT], f16)
        qkw = consts.tile([128, 3, 2 * C], f16)
        vw = consts.tile([128, 3, C], f16)
        pw = consts.tile([128, 3, C], f16)
        band = consts.tile([128, 3, 6, SLAB], f16)
        kmask = consts.tile([1, RT], f16)
        ident = consts.tile([128, 128], f16)
        ones1 = consts.tile([1, 128], f16)
        nc.vector.memset(ones1, 1.0)
        if with_bias:
            pb = consts.tile([1, C], f16)
            nc.sync.dma_start(out=pb, in_=pb_d[:, :])
        # spread input DMAs over four queues; phase-1's first deps come first
        half = RT // 2
        for j in range(3):
            nc.sync.dma_start(out=xT[:, j, 0:half],
                              in_=xT_d[128 * j:128 * (j + 1), 0:half])
            nc.scalar.dma_start(out=xT[:, j, half:RT],
                                in_=xT_d[128 * j:128 * (j + 1), half:RT])
            nc.gpsimd.dma_start(out=qkw[:, j, :], in_=qkw_d[128 * j:128 * (j + 1), :])
        for j in range(3):
            nc.gpsimd.dma_start(out=vw[:, j, :], in_=vw_d[128 * j:128 * (j + 1), :])
            nc.scalar.dma_start(out=pw[:, j, :], in_=pw_d[128 * j:128 * (j + 1), :])
        for v in range(3):
            nc.scalar.dma_start(out=band[:, v, :, :], in_=band_d[v, :, :])
        nc.gpsimd.dma_start(out=kmask, in_=kmask_d[:, :])
        nc.gpsimd.dma_start(out=ident, in_=ident_d[:, :])

        qkT = consts.tile([128, 6, RT], f16)
        # per-head Q with the other head's 64 rows zeroed: lets the score
        # matmuls run at contract-128 / base-0 (walrus splits contract-64
        # matmuls into two HW instructions)
        qkQ = consts.tile([128, 6, RT], f16)
        for par in range(2):
            zsl = qkQ[64 * (1 - par):64 * (2 - par), par, :]
            nc.vector.memset(
                bass.AP(tensor=zsl.tensor, offset=zsl.offset,
                        ap=[zsl.ap[0], [2 * RT, 3], [1, RT]]), 0.0)
        V = consts.tile([128, NVCH, C], f16)
        # ping/pong normalized-score buffers; slot padded to 256 so the
        # 16-wide tail can be DMA-transposed as a full 128-col window
        Phb = []
        for i in range(2):
            Phx = consts.tile([128, 6, 256], f16, tag="Phb%d" % i)
            nc.gpsimd.memset(Phx[:, :, SLAB:256], 0.0)
            Phb.append(Phx)

        # ---- phase 1: qkT = qk_w^T @ x^T  (feature-major) ----
        eng = 0
        with ExitStack() as ph1:
            qkps = ph1.enter_context(tc.tile_pool(name="qkps", bufs=6, space="PSUM"))
            for (n0, ns) in NCH:
                for j in range(6):
                    ps = qkps.tile([128, 512], f32, tag="qkps")
                    for cc in range(3):
                        nc.tensor.matmul(
                            ps[:, :ns],
                            lhsT=qkw[:, cc, 128 * j:128 * (j + 1)],
                            rhs=xT[:, cc, n0:n0 + ns],
                            start=(cc == 0), stop=(cc == 2))
                    if j < 3:
                        nc.vector.tensor_copy(
                            qkQ[0:64, 2 * j, n0:n0 + ns], ps[0:64, :ns])
                        nc.scalar.copy(
                            qkQ[64:128, 2 * j + 1, n0:n0 + ns], ps[64:128, :ns])
                    elif eng % 2 == 0:
                        nc.vector.tensor_copy(qkT[:, j, n0:n0 + ns], ps[:, :ns])
                    else:
                        nc.scalar.copy(qkT[:, j, n0:n0 + ns], ps[:, :ns])
                    eng += 1


        # ---- phase 3: per-block attention + projection ----
        pspool = ctx.enter_context(tc.tile_pool(name="ps8", bufs=1, space="PSUM"))
        PS = pspool.tile([128, 4096], f32)          # the whole PSUM
        ps16 = PS[:, :].bitcast(f16)                # [128, 8192] fp16 view
        sbuf = ctx.enter_context(tc.tile_pool(name="work", bufs=3))

        tiles = {}

        def emit_scores(b, half):
            q0 = w + 128 * b
            k0 = 128 * b
            edge = b in (0, NBLK - 1)
            for h in range(3 * half, 3 * half + 3):
                nc.tensor.matmul(
                    PS[:, 512 * h:512 * h + SLAB],
                    lhsT=qkQ[:, h, q0:q0 + 128],
                    rhs=qkT[:, 3 + h // 2, k0:k0 + SLAB],
                    start=True, stop=True)

        def emit_exp(b, half):
            if half == 0:
                E = sbuf.tile([128, 6, SLAB], f16, tag="E")
                tiles["E", b] = E
            E = tiles["E", b]
            nc.scalar.activation(
                out=E[:, 3 * half:3 * half + 3, :],
                in_=apx(PS[:, :], [[512, 3], [1, SLAB]],
                        extra_off=1536 * half),
                func=AF.Exp)

        def emit_chain(b, half):
            E = tiles["E", b]
            if half == 0:
                P = sbuf.tile([128, 6, SLAB], f16, tag="P")
                sums = sbuf.tile([128, 6], f32, tag="sums")
                recip = sbuf.tile([128, 6], f32, tag="recip")
                tiles["P", b] = P
                tiles["sums", b] = sums
                tiles["recip", b] = recip
                tiles["Ph", b] = Phb[b % 2]
            P = tiles["P", b]
            sums = tiles["sums", b]
            recip = tiles["recip", b]
            Ph = tiles["Ph", b]
            hs = slice(3 * half, 3 * half + 3)
            bv = 0 if b == 0 else (2 if b == NBLK - 1 else 1)
            nc.vector.tensor_mul(P[:, hs, :], E[:, hs, :], band[:, bv, hs, :])
            nc.vector.reduce_sum(out=sums[:, hs], in_=P[:, hs, :], axis=AX.X)
            nc.vector.reciprocal(recip[:, hs], sums[:, hs])
            for h in range(3 * half, 3 * half + 3):
                nc.vector.tensor_scalar_mul(Ph[:, h, 0:SLAB], P[:, h, :],
                                            recip[:, h:h + 1])

        def emit_trans(b):
            Ph = tiles["Ph", b]
            PTa = sbuf.tile([128, 6, 128], f16, tag="PTa")
            Tb = sbuf.tile([16, 6, 128], f16, tag="Tb")
            tiles["PTa", b] = PTa
            tiles["Tb", b] = Tb
            for h in range(6):
                nc.tensor.matmul(
                    ps16[:, 1024 * h:1024 * h + 128],
                    lhsT=Ph[:, h, 0:128], rhs=ident,
                    is_transpose=True, start=True, stop=True)
            nc.vector.tensor_copy(PTa[:, :, :],
                                  apx(ps16[:, 0:128], [[1024, 6], [1, 128]]))
            for wv in range(3):
                for t in range(2):
                    nc.tensor.matmul(
                        ps16[0:W2, 1024 * (6 + t):1024 * (6 + t) + 128],
                        lhsT=Ph[:, 2 * wv + t, 128:128 + W2], rhs=ident,
                        is_transpose=True, start=True, stop=True)
                nc.vector.tensor_copy(
                    Tb[:, 2 * wv:2 * wv + 2, :],
                    apx(ps16[0:16, 0:128], [[1024, 2], [1, 128]],
                        extra_off=6 * 1024))

        def emit_pv_proj(b):
            PTa = tiles.pop(("PTa", b))
            Tb = tiles.pop(("Tb", b))
            tiles.pop(("E", b), None)
            tiles.pop(("Ph", b), None)
            aT = sbuf.tile([128, 3, 128], f16, tag="aT")
            for j in range(3):
                coff = 512 * (6 + (j & 1))
                h0, h1 = 2 * j, 2 * j + 1
                mm0 = nc.tensor.matmul(
                    PS[0:64, coff:coff + 128],
                    lhsT=V[:, b, 64 * h0:64 * h0 + 64],
                    rhs=PTa[:, h0, :], start=True, stop=False)
                mm1 = nc.tensor.matmul(
                    PS[0:64, coff:coff + 128],
                    lhsT=V[0:W2, b + 1, 64 * h0:64 * h0 + 64],
                    rhs=Tb[0:W2, h0, :], start=False, stop=True)
                mm2 = nc.tensor.matmul(
                    PS[64:128, coff:coff + 128],
                    lhsT=V[:, b, 64 * h1:64 * h1 + 64],
                    rhs=PTa[:, h1, :], start=True, stop=False)
                mm3 = nc.tensor.matmul(
                    PS[64:128, coff:coff + 128],
                    lhsT=V[0:W2, b + 1, 64 * h1:64 * h1 + 64],
                    rhs=Tb[0:W2, h1, :], start=False, stop=True)
                add_dep_helper(mm1.ins, mm0.ins, sync=False, reason="grp order")
                add_dep_helper(mm2.ins, mm1.ins, sync=False, reason="grp order")
                add_dep_helper(mm3.ins, mm2.ins, sync=False, reason="grp order")
                if j == 1:
                    nc.vector.tensor_copy(aT[:, j, :], PS[:, coff:coff + 128])
                else:
                    nc.scalar.copy(aT[:, j, :], PS[:, coff:coff + 128])
            yoff = 512 * 7
            for jj in range(3):
                nc.tensor.matmul(
                    PS[:, yoff:yoff + C],
                    lhsT=aT[:, jj, :], rhs=pw[:, jj, :],
                    start=(jj == 0), stop=(jj == 2 and not with_bias))
            if with_bias:
                nc.tensor.matmul(PS[:, yoff:yoff + C],
                                 lhsT=ones1[0:1, :], rhs=pb[0:1, :],
                                 start=False, stop=True)
            Yf = sbuf.tile([128, C], f16, tag="Yf")
            nc.vector.tensor_copy(Yf, PS[:, yoff:yoff + C])
            nc.sync.dma_start(out=out_d[128 * b:128 * (b + 1), :], in_=Yf)

        # 2-stage software pipeline: block b's scores/softmax run while
        # block b-1's transposes/PV/projection drain on banks 6,7
        def emit_v(r):
            rlen = min(128, RT - 128 * r)
            voff = 512 * (6 + (r & 1))
            for cc in range(3):
                nc.tensor.matmul(
                    PS[:rlen, voff:voff + C],
                    lhsT=xT[:, cc, 128 * r:128 * r + rlen],
                    rhs=vw[:, cc, :],
                    start=(cc == 0), stop=(cc == 2))
            if r % 2 == 0:
                nc.vector.tensor_copy(V[:rlen, r, :], PS[:rlen, voff:voff + C])
            else:
                nc.scalar.copy(V[:rlen, r, :], PS[:rlen, voff:voff + C])

        for b in range(NBLK):
            if b >= 1:
                emit_trans(b - 1)
            emit_scores(b, 0)
            emit_exp(b, 0)
            emit_scores(b, 1)
            emit_chain(b, 0)
            emit_exp(b, 1)
            if b == 0:
                for r in range(NVCH):
                    emit_v(r)
            if b >= 1:
                emit_pv_proj(b - 1)
            emit_chain(b, 1)
        emit_trans(NBLK - 1)
        emit_pv_proj(NBLK - 1)

    nc.finalize()
    return nc


def make_in_maps(x, qkv_w, proj_w, proj_b, w, with_bias):
    W2 = 2 * w
    RT = R + W2
    SLAB = 128 + W2

    qk_w = qkv_w[:2 * C].copy()
    qk_w[:C] *= SCALE                       # fold softmax scale into Q weights
    qk_wT = np.ascontiguousarray(qk_w.T).astype(np.float16)
    v_wT = np.ascontiguousarray(qkv_w[2 * C:].T).astype(np.float16)
    proj_wT = np.ascontiguousarray(proj_w.T).astype(np.float16)
    pb16 = proj_b.reshape(1, C).astype(np.float16)
    ident = np.eye(128, dtype=np.float16)

    p = np.arange(128)[:, None]
    t = np.arange(SLAB)[None, :]
    band = ((t >= p) & (t <= p + W2)).astype(np.float16)

    in_maps = []
    for c in range(NCORES):
        b, s = divmod(c, SEQ_SHARDS)
        g0 = s * R
        xpad = np.zeros((RT, C), dtype=np.float32)
        lo = max(0, g0 - w)
        hi = min(N, g0 + R + w)
        xpad[lo - (g0 - w): hi - (g0 - w)] = x[b, lo:hi]
        xT = np.ascontiguousarray(xpad.T).astype(np.float16)
        kmask = np.zeros((1, RT), dtype=np.float16)
        bandF = band.copy()
        bandL = band.copy()
        if s == 0:
            bandF[:, :w] = 0.0          # keys with global j < 0
        if s == SEQ_SHARDS - 1:
            bandL[:, SLAB - w:] = 0.0   # keys with global j >= N
        band3 = np.stack([np.tile(bv, (1, 6)) for bv in (bandF, band, bandL)])
        m = {"xT": xT, "qk_wT": qk_wT, "v_wT": v_wT, "proj_wT": proj_wT,
             "band": band3, "kmask": kmask, "ident": ident}
        if with_bias:
            m["proj_b16"] = pb16
        in_maps.append(m)
    return in_maps


def kernel(x, qkv_w, proj_w, proj_b, epoch):
    x = np.asarray(x, dtype=np.float32)
    qkv_w = np.asarray(qkv_w, dtype=np.float32)
    proj_w = np.asarray(proj_w, dtype=np.float32)
    proj_b = np.asarray(proj_b, dtype=np.float32)
    w = _window_for_epoch(int(np.asarray(epoch)))
    if w is None or 128 + 2 * w > 512:
        return _numpy_fallback(x, qkv_w, proj_w, proj_b, w)

    from concourse.bass_utils import run_bass_kernel_spmd

    with_bias = bool(np.any(proj_b != 0.0))
    key = (w, with_bias)
    if key not in _PROGRAM_CACHE:
        _PROGRAM_CACHE[key] = build_program(w, with_bias)
    nc = _PROGRAM_CACHE[key]

    in_maps = make_in_maps(x, qkv_w, proj_w, proj_b, w, with_bias)
    res = run_bass_kernel_spmd(nc, in_maps, core_ids=list(range(NCORES)))

    out = np.empty((B, N, C), dtype=np.float32)
    for c in range(NCORES):
        b, s = divmod(c, SEQ_SHARDS)
        out[b, s * R:(s + 1) * R] = res.results[c]["out"].astype(np.float32)
    return out
